# revision 36
# baseline (speedup 1.0000x reference)
"""Trainium2 Bass kernel for CPELayer_ResAG (concept-routed LoRA edit layer).

Computation (per token t with concept c = concept_idx[t]):
    down = edit_direction[t] @ lora_down[c]          # [768]@[768,4] -> [4]
    up   = down @ lora_up[c]                         # [4]@[4,1280]  -> [1280]
    out  = x[t] @ W.T + b_lin + 0.25 * up

Strategy: data-parallel over batch across 8 cores (616 tokens/core).
The routed LoRA is computed densely: A.T[(c,r), t] = lora_down_flat.T @ ed.T
for ALL concepts (only ~6% extra PE work), then masked on-device with a
one-hot built by DVE is_equal (the MoE routing), and contracted back with
lora_up_flat via the tensor engine, accumulating into the same PSUM as the
org matmul.  The bias is folded in as one extra contraction row (ones row in
the masked operand, b_lin row in the lora_up operand).  The 0.25 LoRA scale
is folded into lora_up host-side (exact: power of two).

v2 (all-bf16, overlap-tuned): every operand and the output travel as bf16
(abs rel err ~4e-3, budget 2e-2), halving HBM traffic vs fp32r. x.T and W.T
are packed host-side into one [128, 6, 616+1280] tensor so each k-tile of
the d_in contraction arrives as ONE contiguous-line DMA and the org matmuls
track per-k arrival.  A dozen dummy matmuls on a memset tile run during the
load phase so the PE clock (DVFS: 0.65/1.2/2.4 GHz ramp) is at full rate
when real work starts.  DMA issues are split across the two hardware DGE
queues (sync + scalar).  The concept-id compare column rides in the lora_up
tensor (col 1280) instead of a separate DMA.
"""

import sys
import types

import numpy as np

import concourse.mybir as mybir
import concourse.tile as tile
from concourse import bacc
from concourse.bass_utils import run_bass_kernel_spmd

# If BASS_TRACE is set in the environment, run_bass_kernel_spmd imports
# antenv.axon_hooks, which some containers lack; stub it (None hook ->
# tracing is skipped gracefully, execution unaffected).
try:
    import antenv.axon_hooks  # noqa: F401
except ImportError:
    _m = types.ModuleType("antenv.axon_hooks")
    _m.get_axon_ntff_profile_hook = lambda: None
    _m.set_axon_ntff_profile_hook = lambda h: None
    sys.modules["antenv.axon_hooks"] = _m

# Problem shapes (hardcoded per spec nn_CPELayer_ResAG_19335942766951)
N_CORES = 8
B, T, DIN, DOUT = 64, 77, 768, 1280
N_CONCEPTS, RANK = 50, 4
SCALE = 0.25  # alpha/rank = 1/4, exact power of two
BPC = B // N_CORES          # batches per core = 8
TOK = BPC * T               # tokens per core = 616
NJ = N_CONCEPTS * RANK      # 200 flattened (concept, rank) rows
KJ_PAD = 256                # padded rows: 200 lora + 1 bias + 55 zero
P = 128
KD = DIN // P               # 6 k-tiles of the d_in contraction
KH = KD // 2                # ed arrives in two halves of 3 k-tiles
NH = 308                    # half of TOK for the A.T psum tiles
XW = TOK + DOUT             # combined x.T | W.T free width = 1896
LUW = DOUT + 1              # lora_up width + concept-id compare column
T_EDGES = [0, 128, 256, 384, 512, 616]
N_CHUNKS = [(0, 512), (512, 512), (1024, 256)]
N_WARM = 13                 # dummy matmuls to ramp the PE clock; sized so
                            # the warm block ends right as ed_a/ld land
                            # (any idle gap resets the DVFS ramp, and the
                            # later warms run 2x faster once it hits full)
KA = 2                      # org k-tiles joining the up-matmuls in wave A

_cache = {}


def _build_bass(out_f32=False):
    nc = bacc.Bacc("TRN2", target_bir_lowering=False, debug=False,
                   num_devices=N_CORES)
    f32 = mybir.dt.float32
    bf16 = mybir.dt.bfloat16
    odt = f32 if out_f32 else bf16

    xw_d = nc.dram_tensor("xw", [P, KD, XW], bf16, kind="ExternalInput").ap()
    edT_d = nc.dram_tensor("edT", [P, KD, TOK], bf16,
                           kind="ExternalInput").ap()
    idx_d = nc.dram_tensor("idxf", [1, TOK], bf16, kind="ExternalInput").ap()
    ldT_d = nc.dram_tensor("ldT", [P, KD, NJ], bf16,
                           kind="ExternalInput").ap()
    lu_d = nc.dram_tensor("luB", [P, 2, LUW], bf16, kind="ExternalInput").ap()
    out_d = nc.dram_tensor("out", [TOK, DOUT], odt, kind="ExternalOutput").ap()

    with tile.TileContext(nc) as tc:
        with (
            tc.tile_pool(name="consts", bufs=1) as consts,
            tc.tile_pool(name="osb32", bufs=5) as osb32p,
            tc.tile_pool(name="osbbf", bufs=5) as osbbfp,
        ):
            # Warm-up source: zeros tile the dummy matmuls stream over while
            # the first DMAs are in flight (PE DVFS ramps after ~3us busy).
            wsrc = consts.tile([P, NH], bf16, tag="wsrc")
            nc.gpsimd.memset(wsrc[:], 0.0)

            # MT holds the masked (routed) A.T rows.  Chunk-1 rows 72..127
            # pair with luB rows 200..255: rows 64..128 zeroed, then the
            # ones row at 96 (bias: b_lin sits at luB row 224); the mask-mul
            # below overwrites rows 0..71 (lora j=128..199).
            MT = []
            for jc in (0, 1):
                mt_t = consts.tile([P, TOK], bf16, tag=f"MT{jc}")
                MT.append(mt_t)
            nc.gpsimd.memset(MT[1][64:P, :], 0.0)
            nc.gpsimd.memset(MT[1][96:97, :], 1.0)

            # DMA issues: ~0.7us each of engine-queue time, so they are
            # split across the two hardware DGE queues (sync + scalar) and
            # ordered by need: the A.T chain (ed/ld) first on sync, the
            # small routing tensors on scalar, then the org k-tiles.  idx
            # goes over as one 2.4KB line and is partition-broadcast by the
            # otherwise-idle gpsimd so it doesn't steal HBM bandwidth from
            # ed_a during the critical load window.
            # Bulk inputs ride the sync ring in need-order; the small idx
            # broadcast rides the scalar ring concurrently so the mask
            # chain (idx -> is_equal -> MT mult) completes before the A.T
            # matmuls do, never gating wave A.
            # ed_a is the head's critical tensor: its first two k-tiles go
            # on the sync ring while k2 + the small routing tensors ride
            # the scalar ring concurrently, so the A.T chain starts ~3us
            # sooner than a single-ring schedule allows.
            ed_a = consts.tile([P, KH, TOK], bf16, tag="ed_a")
            nc.sync.dma_start(ed_a[:, 0:2, :], edT_d[:, 0:2, :])
            nc.scalar.dma_start(ed_a[:, 2:KH, :], edT_d[:, 2:KH, :])
            ld_all = consts.tile([P, KD, NJ], bf16, tag="ld_all")
            nc.scalar.dma_start(ld_all[:], ldT_d[:, :, :])
            idx_bc = consts.tile([P, TOK], bf16, tag="idx_bc")
            nc.scalar.dma_start(idx_bc[:], idx_d.partition_broadcast(P))
            lu_all = consts.tile([P, 2, LUW], bf16, tag="lu_all")
            nc.scalar.dma_start(lu_all[:], lu_d[:, :, :])
            ed_b = consts.tile([P, KD - KH, TOK], bf16, tag="ed_b")
            nc.sync.dma_start(ed_b[:], edT_d[:, KH:KD, :])
            xw = []
            for k in range(KD):
                t_ = consts.tile([P, XW], bf16, tag=f"xw{k}")
                nc.sync.dma_start(t_[:], xw_d[:, k, :])
                xw.append(t_)

            edT = ([ed_a[:, k, :] for k in range(KH)]
                   + [ed_b[:, k, :] for k in range(KD - KH)])
            xT = [xw[k][:, 0:TOK] for k in range(KD)]
            WT = [xw[k][:, TOK:XW] for k in range(KD)]
            lu = [lu_all[:, j, 0:DOUT] for j in range(2)]
            # is_equal needs an f32 per-partition scalar; the compare column
            # rides in the bf16 lu tensor, so up-convert the 2 values first.
            cv32 = consts.tile([P, 2], f32, tag="cv32")
            nc.vector.tensor_copy(out=cv32[:], in_=lu_all[:, :, DOUT])
            cv = [cv32[:, j:j + 1] for j in range(2)]

            # One-hot routing masks, built by DVE while DMAs stream.
            masks = []
            for jc in range(2):
                m = consts.tile([P, TOK], f32, tag=f"mask{jc}")
                nc.vector.tensor_scalar(
                    m[:], idx_bc[:], cv[jc], None, mybir.AluOpType.is_equal)
                masks.append(m)

            with tc.tile_pool(name="at_ps", bufs=4, space="PSUM") as at_pool, \
                    tc.tile_pool(name="out_ps", bufs=4,
                                 space="PSUM") as out_pool:
                # PE clock warm-up: ~3us of zero matmuls on one psum bank
                # (shares the "at" tag/slots; it retires before the 5th
                # at-tile allocation needs the slot back).
                warm = at_pool.tile([P, NH], f32, tag="at")
                for _ in range(N_WARM):
                    nc.tensor.matmul(warm[:], wsrc[:, 0:P], wsrc[:],
                                     start=True, stop=True)

                # A.T[(c,r), t] = lora_down_flat.T @ ed.T for all concepts,
                # k-outer so k0..2 stream on ed_a while ed_b is in flight;
                # masked into MT (the routed "down" activations, transposed).
                ats = {}
                for jc in range(2):
                    jp = P if jc == 0 else NJ - P  # 128, 72
                    for nh in range(2):
                        at_t = at_pool.tile([P, NH], f32, tag="at")
                        ats[(jc, nh)] = at_t
                midwarm = out_pool.tile([P, 512], f32, tag="ops")
                for k in range(KD):
                    if k == KH:
                        for _ in range(3):
                            nc.tensor.matmul(midwarm[:, 0:NH],
                                             wsrc[:, 0:P], wsrc[:],
                                             start=True, stop=True)
                    for jc in range(2):
                        jp = P if jc == 0 else NJ - P
                        jsl = slice(jc * P, jc * P + jp)
                        for nh in range(2):
                            nsl = slice(nh * NH, (nh + 1) * NH)
                            nc.tensor.matmul(
                                ats[(jc, nh)][:jp, :], ld_all[:, k, jsl],
                                edT[k][:, nsl], start=(k == 0),
                                stop=(k == KD - 1))
                for nh in range(2):
                    nsl = slice(nh * NH, (nh + 1) * NH)
                    for jc in range(2):
                        jp = P if jc == 0 else NJ - P
                        nc.vector.tensor_tensor(
                            MT[jc][:jp, nsl], ats[(jc, nh)][:jp, :],
                            masks[jc][:jp, nsl], mybir.AluOpType.mult)

                # Main accumulation, two short-lived PSUM waves per (t, n) so
                # banks recycle while the xw k-tiles stream in.  Wave A only
                # needs MT/lu (+ xw k<KA), which arrive early; it buys ~6us
                # of PE time during which the remaining k-tiles stream in,
                # so wave B never waits on the DMA ring:
                #   wave A: up1+up2 + org k<KA -> copy to osb32
                #   wave B: org k=KA..5 -> DVE-add (bf16 out) -> DMA out
                osb32s = []
                gi = 0
                for ti in range(len(T_EDGES) - 1):
                    t0, t1 = T_EDGES[ti], T_EDGES[ti + 1]
                    tw = t1 - t0
                    tsl = slice(t0, t1)
                    osb = osb32p.tile([P, DOUT], f32, tag="osb32")
                    osb32s.append(osb)
                    for (n0, nw) in N_CHUNKS:
                        ps = out_pool.tile([P, 512], f32, tag="ops")
                        nmm = 2 + KA
                        i = 0
                        for jc in range(2):
                            nc.tensor.matmul(
                                ps[:tw, :nw], MT[jc][:, tsl],
                                lu[jc][:, n0:n0 + nw],
                                start=(i == 0), stop=(i == nmm - 1))
                            i += 1
                        for k in range(KA):
                            nc.tensor.matmul(
                                ps[:tw, :nw], xT[k][:, tsl],
                                WT[k][:, n0:n0 + nw],
                                start=(i == 0), stop=(i == nmm - 1))
                            i += 1
                        # alternate the copy engine so psum banks recycle
                        # at matmul pace
                        if gi % 2 == 0:
                            nc.scalar.copy(osb[:tw, n0:n0 + nw],
                                           ps[:tw, :nw])
                        else:
                            nc.vector.tensor_copy(out=osb[:tw, n0:n0 + nw],
                                                  in_=ps[:tw, :nw])
                        gi += 1
                # wave B runs the t-tiles in reverse so the short 104-row
                # tile drains early and the LAST tile's store is split per
                # n-chunk across both rings, minimizing the final chain
                n_t = len(T_EDGES) - 1
                for wi, ti in enumerate(reversed(range(n_t))):
                    t0, t1 = T_EDGES[ti], T_EDGES[ti + 1]
                    tw = t1 - t0
                    tsl = slice(t0, t1)
                    osb = osb32s[ti]
                    obf = osbbfp.tile([P, DOUT], odt, tag="osbbf")
                    last = (wi == n_t - 1)
                    for ni, (n0, nw) in enumerate(N_CHUNKS):
                        ps = out_pool.tile([P, 512], f32, tag="ops")
                        for i, k in enumerate(range(KA, KD)):
                            nc.tensor.matmul(
                                ps[:tw, :nw], xT[k][:, tsl],
                                WT[k][:, n0:n0 + nw],
                                start=(i == 0), stop=(i == KD - KA - 1))
                        nc.vector.tensor_tensor(
                            obf[:tw, n0:n0 + nw], ps[:tw, :nw],
                            osb[:tw, n0:n0 + nw], mybir.AluOpType.add)
                        if last:
                            oeng = nc.scalar if ni % 2 == 0 else nc.sync
                            oeng.dma_start(out_d[tsl, n0:n0 + nw],
                                           obf[:tw, n0:n0 + nw])
                    if not last:
                        # out tiles alternate rings so the ~2us-per-tile
                        # store stream runs 2-wide under the compute
                        oeng = nc.scalar if wi % 2 == 0 else nc.sync
                        oeng.dma_start(out_d[tsl, :], obf[:tw, :])

    nc.compile()
    return nc


def get_bass(out_f32=False):
    key = bool(out_f32)
    if key not in _cache:
        _cache[key] = _build_bass(out_f32)
    return _cache[key]


def make_in_maps(x, edit_direction, concept_idx, lora_down, lora_up, W,
                 b_lin):
    """Host-side sharding + layout prep (no reference FLOPs)."""
    bf = mybir.dt.np(mybir.dt.bfloat16)
    x = np.asarray(x, dtype=np.float32)
    ed = np.asarray(edit_direction, dtype=np.float32)
    idx = np.asarray(concept_idx)
    ld = np.asarray(lora_down, dtype=np.float32)
    lup = np.asarray(lora_up, dtype=np.float32)
    W = np.asarray(W, dtype=np.float32)
    b = np.asarray(b_lin, dtype=np.float32)

    # W.T as [128, 6, 1280] (k-tiles of d_in side by side per partition)
    WTk = np.ascontiguousarray(
        W.T.reshape(KD, P, DOUT).transpose(1, 0, 2).astype(bf))
    ldT = np.ascontiguousarray(
        ld.transpose(1, 0, 2).reshape(KD, P, NJ).transpose(1, 0, 2)
        .astype(bf))
    luB = np.zeros((KJ_PAD, LUW), dtype=np.float32)
    luB[:NJ, :DOUT] = lup.reshape(NJ, DOUT) * SCALE            # exact x0.25
    luB[128 + 96, :DOUT] = b                                   # bias row
    cvf = np.full(KJ_PAD, -1.0, dtype=np.float32)
    cvf[:NJ] = np.arange(NJ, dtype=np.float32) // RANK
    luB[:, DOUT] = cvf                                         # compare col
    luB = np.ascontiguousarray(
        luB.reshape(2, P, LUW).transpose(1, 0, 2).astype(bf))

    in_maps = []
    for c in range(N_CORES):
        sl = slice(c * BPC, (c + 1) * BPC)
        xs = x[sl].reshape(TOK, DIN)
        eds = ed[sl].reshape(TOK, DIN)
        idxs = idx[sl].reshape(TOK).astype(np.float32)
        xTk = xs.T.reshape(KD, P, TOK).transpose(1, 0, 2).astype(bf)
        xwc = np.concatenate([xTk, WTk], axis=2)               # [128,6,1896]
        edk = np.ascontiguousarray(
            eds.T.reshape(KD, P, TOK).transpose(1, 0, 2).astype(bf))
        in_maps.append({
            "xw": np.ascontiguousarray(xwc),
            "edT": edk,
            "idxf": np.ascontiguousarray(idxs.reshape(1, TOK).astype(bf)),
            "ldT": ldT,
            "luB": luB,
        })
    return in_maps


def kernel(x, edit_direction, concept_idx, lora_down, lora_up, W, b_lin,
           _trace=False, _out_f32=False, **_ignored):
    nc = get_bass(_out_f32)
    in_maps = make_in_maps(x, edit_direction, concept_idx, lora_down,
                           lora_up, W, b_lin)
    res = run_bass_kernel_spmd(nc, in_maps, core_ids=list(range(N_CORES)),
                               trace=_trace)
    out = np.concatenate(
        [np.asarray(r["out"], dtype=np.float32) for r in res.results], axis=0)
    out = out.reshape(B, T, DOUT)
    if _trace:
        kernel.last_results = res
    return out


# revision 37
# speedup vs baseline: 1.0007x; 1.0007x over previous
"""Trainium2 Bass kernel for CPELayer_ResAG (concept-routed LoRA edit layer).

Computation (per token t with concept c = concept_idx[t]):
    down = edit_direction[t] @ lora_down[c]          # [768]@[768,4] -> [4]
    up   = down @ lora_up[c]                         # [4]@[4,1280]  -> [1280]
    out  = x[t] @ W.T + b_lin + 0.25 * up

Strategy: data-parallel over batch across 8 cores (616 tokens/core).
The routed LoRA is computed densely: A.T[(c,r), t] = lora_down_flat.T @ ed.T
for ALL concepts (only ~6% extra PE work), then masked on-device with a
one-hot built by DVE is_equal (the MoE routing), and contracted back with
lora_up_flat via the tensor engine, accumulating into the same PSUM as the
org matmul.  The bias is folded in as one extra contraction row (ones row in
the masked operand, b_lin row in the lora_up operand).  The 0.25 LoRA scale
is folded into lora_up host-side (exact: power of two).

v2 (all-bf16, overlap-tuned): every operand and the output travel as bf16
(abs rel err ~4e-3, budget 2e-2), halving HBM traffic vs fp32r. x.T and W.T
are packed host-side into one [128, 6, 616+1280] tensor so each k-tile of
the d_in contraction arrives as ONE contiguous-line DMA and the org matmuls
track per-k arrival.  A dozen dummy matmuls on a memset tile run during the
load phase so the PE clock (DVFS: 0.65/1.2/2.4 GHz ramp) is at full rate
when real work starts.  DMA issues are split across the two hardware DGE
queues (sync + scalar).  The concept-id compare column rides in the lora_up
tensor (col 1280) instead of a separate DMA.
"""

import sys
import types

import numpy as np

import concourse.mybir as mybir
import concourse.tile as tile
from concourse import bacc
from concourse.bass_utils import run_bass_kernel_spmd

# If BASS_TRACE is set in the environment, run_bass_kernel_spmd imports
# antenv.axon_hooks, which some containers lack; stub it (None hook ->
# tracing is skipped gracefully, execution unaffected).
try:
    import antenv.axon_hooks  # noqa: F401
except ImportError:
    _m = types.ModuleType("antenv.axon_hooks")
    _m.get_axon_ntff_profile_hook = lambda: None
    _m.set_axon_ntff_profile_hook = lambda h: None
    sys.modules["antenv.axon_hooks"] = _m

# Problem shapes (hardcoded per spec nn_CPELayer_ResAG_19335942766951)
N_CORES = 8
B, T, DIN, DOUT = 64, 77, 768, 1280
N_CONCEPTS, RANK = 50, 4
SCALE = 0.25  # alpha/rank = 1/4, exact power of two
BPC = B // N_CORES          # batches per core = 8
TOK = BPC * T               # tokens per core = 616
NJ = N_CONCEPTS * RANK      # 200 flattened (concept, rank) rows
KJ_PAD = 256                # padded rows: 200 lora + 1 bias + 55 zero
P = 128
KD = DIN // P               # 6 k-tiles of the d_in contraction
KH = KD // 2                # ed arrives in two halves of 3 k-tiles
NH = 308                    # half of TOK for the A.T psum tiles
XW = TOK + DOUT             # combined x.T | W.T free width = 1896
LUW = DOUT + 1              # lora_up width + concept-id compare column
T_EDGES = [0, 128, 256, 384, 512, 616]
N_CHUNKS = [(0, 512), (512, 512), (1024, 256)]
N_WARM = 14                 # dummy matmuls to ramp the PE clock; sized so
                            # the warm block ends right as ed_a/ld land
                            # (any idle gap resets the DVFS ramp, and the
                            # later warms run 2x faster once it hits full)
KA = 2                      # org k-tiles joining the up-matmuls in wave A

_cache = {}


def _build_bass(out_f32=False):
    nc = bacc.Bacc("TRN2", target_bir_lowering=False, debug=False,
                   num_devices=N_CORES)
    f32 = mybir.dt.float32
    bf16 = mybir.dt.bfloat16
    odt = f32 if out_f32 else bf16

    xw_d = nc.dram_tensor("xw", [P, KD, XW], bf16, kind="ExternalInput").ap()
    edT_d = nc.dram_tensor("edT", [P, KD, TOK], bf16,
                           kind="ExternalInput").ap()
    idx_d = nc.dram_tensor("idxf", [1, TOK], bf16, kind="ExternalInput").ap()
    ldT_d = nc.dram_tensor("ldT", [P, KD, NJ], bf16,
                           kind="ExternalInput").ap()
    lu_d = nc.dram_tensor("luB", [P, 2, LUW], bf16, kind="ExternalInput").ap()
    out_d = nc.dram_tensor("out", [TOK, DOUT], odt, kind="ExternalOutput").ap()

    with tile.TileContext(nc) as tc:
        with (
            tc.tile_pool(name="consts", bufs=1) as consts,
            tc.tile_pool(name="osb32", bufs=5) as osb32p,
            tc.tile_pool(name="osbbf", bufs=5) as osbbfp,
        ):
            # Warm-up source: zeros tile the dummy matmuls stream over while
            # the first DMAs are in flight (PE DVFS ramps after ~3us busy).
            wsrc = consts.tile([P, NH], bf16, tag="wsrc")
            nc.gpsimd.memset(wsrc[:], 0.0)

            # MT holds the masked (routed) A.T rows.  Chunk-1 rows 72..127
            # pair with luB rows 200..255: rows 64..128 zeroed, then the
            # ones row at 96 (bias: b_lin sits at luB row 224); the mask-mul
            # below overwrites rows 0..71 (lora j=128..199).
            MT = []
            for jc in (0, 1):
                mt_t = consts.tile([P, TOK], bf16, tag=f"MT{jc}")
                MT.append(mt_t)
            nc.gpsimd.memset(MT[1][64:P, :], 0.0)
            nc.gpsimd.memset(MT[1][96:97, :], 1.0)

            # DMA issues: ~0.7us each of engine-queue time, so they are
            # split across the two hardware DGE queues (sync + scalar) and
            # ordered by need: the A.T chain (ed/ld) first on sync, the
            # small routing tensors on scalar, then the org k-tiles.  idx
            # goes over as one 2.4KB line and is partition-broadcast by the
            # otherwise-idle gpsimd so it doesn't steal HBM bandwidth from
            # ed_a during the critical load window.
            # Bulk inputs ride the sync ring in need-order; the small idx
            # broadcast rides the scalar ring concurrently so the mask
            # chain (idx -> is_equal -> MT mult) completes before the A.T
            # matmuls do, never gating wave A.
            # Ring throughput scales with packet size (per-packet latency
            # ~0.2us, ~16 in flight), so ed keeps its 3696B-run layout and
            # the sync ring carries ONLY the big-packet tensor-engine
            # stream in need-order (ed_a, ed_b, xw k-tiles) while the
            # small routing tensors (ld/idx/lu) ride the scalar ring in
            # parallel.
            ed_a = consts.tile([P, KH, TOK], bf16, tag="ed_a")
            nc.sync.dma_start(ed_a[:], edT_d[:, 0:KH, :])
            ed_b = consts.tile([P, KD - KH, TOK], bf16, tag="ed_b")
            nc.sync.dma_start(ed_b[:], edT_d[:, KH:KD, :])
            ld_all = consts.tile([P, KD, NJ], bf16, tag="ld_all")
            nc.scalar.dma_start(ld_all[:], ldT_d[:, :, :])
            idx_bc = consts.tile([P, TOK], bf16, tag="idx_bc")
            nc.scalar.dma_start(idx_bc[:], idx_d.partition_broadcast(P))
            lu_all = consts.tile([P, 2, LUW], bf16, tag="lu_all")
            nc.scalar.dma_start(lu_all[:], lu_d[:, :, :])
            xw = []
            for k in range(KD):
                t_ = consts.tile([P, XW], bf16, tag=f"xw{k}")
                nc.sync.dma_start(t_[:], xw_d[:, k, :])
                xw.append(t_)

            edT = ([ed_a[:, k, :] for k in range(KH)]
                   + [ed_b[:, k, :] for k in range(KD - KH)])
            xT = [xw[k][:, 0:TOK] for k in range(KD)]
            WT = [xw[k][:, TOK:XW] for k in range(KD)]
            lu = [lu_all[:, j, 0:DOUT] for j in range(2)]
            # is_equal needs an f32 per-partition scalar; the compare column
            # rides in the bf16 lu tensor, so up-convert the 2 values first.
            cv32 = consts.tile([P, 2], f32, tag="cv32")
            nc.vector.tensor_copy(out=cv32[:], in_=lu_all[:, :, DOUT])
            cv = [cv32[:, j:j + 1] for j in range(2)]

            # One-hot routing masks, built by DVE while DMAs stream.
            masks = []
            for jc in range(2):
                m = consts.tile([P, TOK], f32, tag=f"mask{jc}")
                nc.vector.tensor_scalar(
                    m[:], idx_bc[:], cv[jc], None, mybir.AluOpType.is_equal)
                masks.append(m)

            with tc.tile_pool(name="at_ps", bufs=4, space="PSUM") as at_pool, \
                    tc.tile_pool(name="out_ps", bufs=4,
                                 space="PSUM") as out_pool:
                # PE clock warm-up: ~3us of zero matmuls on one psum bank
                # (shares the "at" tag/slots; it retires before the 5th
                # at-tile allocation needs the slot back).
                warm = at_pool.tile([P, NH], f32, tag="at")
                for _ in range(N_WARM):
                    nc.tensor.matmul(warm[:], wsrc[:, 0:P], wsrc[:],
                                     start=True, stop=True)

                # A.T[(c,r), t] = lora_down_flat.T @ ed.T for all concepts,
                # k-outer so k0..2 stream on ed_a while ed_b is in flight;
                # masked into MT (the routed "down" activations, transposed).
                ats = {}
                for jc in range(2):
                    jp = P if jc == 0 else NJ - P  # 128, 72
                    for nh in range(2):
                        at_t = at_pool.tile([P, NH], f32, tag="at")
                        ats[(jc, nh)] = at_t
                midwarm = out_pool.tile([P, 512], f32, tag="ops")
                for k in range(KD):
                    if k == KH:
                        for _ in range(3):
                            nc.tensor.matmul(midwarm[:, 0:NH],
                                             wsrc[:, 0:P], wsrc[:],
                                             start=True, stop=True)
                    for jc in range(2):
                        jp = P if jc == 0 else NJ - P
                        jsl = slice(jc * P, jc * P + jp)
                        for nh in range(2):
                            nsl = slice(nh * NH, (nh + 1) * NH)
                            nc.tensor.matmul(
                                ats[(jc, nh)][:jp, :], ld_all[:, k, jsl],
                                edT[k][:, nsl], start=(k == 0),
                                stop=(k == KD - 1))
                for nh in range(2):
                    nsl = slice(nh * NH, (nh + 1) * NH)
                    for jc in range(2):
                        jp = P if jc == 0 else NJ - P
                        nc.vector.tensor_tensor(
                            MT[jc][:jp, nsl], ats[(jc, nh)][:jp, :],
                            masks[jc][:jp, nsl], mybir.AluOpType.mult)

                # Main accumulation, two short-lived PSUM waves per (t, n) so
                # banks recycle while the xw k-tiles stream in.  Wave A only
                # needs MT/lu (+ xw k<KA), which arrive early; it buys ~6us
                # of PE time during which the remaining k-tiles stream in,
                # so wave B never waits on the DMA ring:
                #   wave A: up1+up2 + org k<KA -> copy to osb32
                #   wave B: org k=KA..5 -> DVE-add (bf16 out) -> DMA out
                osb32s = []
                gi = 0
                for ti in range(len(T_EDGES) - 1):
                    t0, t1 = T_EDGES[ti], T_EDGES[ti + 1]
                    tw = t1 - t0
                    tsl = slice(t0, t1)
                    osb = osb32p.tile([P, DOUT], f32, tag="osb32")
                    osb32s.append(osb)
                    for (n0, nw) in N_CHUNKS:
                        ps = out_pool.tile([P, 512], f32, tag="ops")
                        nmm = 2 + KA
                        i = 0
                        for jc in range(2):
                            nc.tensor.matmul(
                                ps[:tw, :nw], MT[jc][:, tsl],
                                lu[jc][:, n0:n0 + nw],
                                start=(i == 0), stop=(i == nmm - 1))
                            i += 1
                        for k in range(KA):
                            nc.tensor.matmul(
                                ps[:tw, :nw], xT[k][:, tsl],
                                WT[k][:, n0:n0 + nw],
                                start=(i == 0), stop=(i == nmm - 1))
                            i += 1
                        # alternate the copy engine so psum banks recycle
                        # at matmul pace
                        if gi % 2 == 0:
                            nc.scalar.copy(osb[:tw, n0:n0 + nw],
                                           ps[:tw, :nw])
                        else:
                            nc.vector.tensor_copy(out=osb[:tw, n0:n0 + nw],
                                                  in_=ps[:tw, :nw])
                        gi += 1
                # wave B runs the t-tiles in reverse so the short 104-row
                # tile drains early and the LAST tile's store is split per
                # n-chunk across both rings, minimizing the final chain
                n_t = len(T_EDGES) - 1
                for wi, ti in enumerate(reversed(range(n_t))):
                    t0, t1 = T_EDGES[ti], T_EDGES[ti + 1]
                    tw = t1 - t0
                    tsl = slice(t0, t1)
                    osb = osb32s[ti]
                    obf = osbbfp.tile([P, DOUT], odt, tag="osbbf")
                    last = (wi == n_t - 1)
                    for ni, (n0, nw) in enumerate(N_CHUNKS):
                        ps = out_pool.tile([P, 512], f32, tag="ops")
                        for i, k in enumerate(range(KA, KD)):
                            nc.tensor.matmul(
                                ps[:tw, :nw], xT[k][:, tsl],
                                WT[k][:, n0:n0 + nw],
                                start=(i == 0), stop=(i == KD - KA - 1))
                        nc.vector.tensor_tensor(
                            obf[:tw, n0:n0 + nw], ps[:tw, :nw],
                            osb[:tw, n0:n0 + nw], mybir.AluOpType.add)
                        if last:
                            oeng = nc.scalar if ni % 2 == 0 else nc.sync
                            oeng.dma_start(out_d[tsl, n0:n0 + nw],
                                           obf[:tw, n0:n0 + nw])
                    if not last:
                        # out tiles alternate rings so the ~2us-per-tile
                        # store stream runs 2-wide under the compute
                        oeng = nc.scalar if wi % 2 == 0 else nc.sync
                        oeng.dma_start(out_d[tsl, :], obf[:tw, :])

    nc.compile()
    return nc


def get_bass(out_f32=False):
    key = bool(out_f32)
    if key not in _cache:
        _cache[key] = _build_bass(out_f32)
    return _cache[key]


def make_in_maps(x, edit_direction, concept_idx, lora_down, lora_up, W,
                 b_lin):
    """Host-side sharding + layout prep (no reference FLOPs)."""
    bf = mybir.dt.np(mybir.dt.bfloat16)
    x = np.asarray(x, dtype=np.float32)
    ed = np.asarray(edit_direction, dtype=np.float32)
    idx = np.asarray(concept_idx)
    ld = np.asarray(lora_down, dtype=np.float32)
    lup = np.asarray(lora_up, dtype=np.float32)
    W = np.asarray(W, dtype=np.float32)
    b = np.asarray(b_lin, dtype=np.float32)

    # W.T as [128, 6, 1280] (k-tiles of d_in side by side per partition)
    WTk = np.ascontiguousarray(
        W.T.reshape(KD, P, DOUT).transpose(1, 0, 2).astype(bf))
    ldT = np.ascontiguousarray(
        ld.transpose(1, 0, 2).reshape(KD, P, NJ).transpose(1, 0, 2)
        .astype(bf))
    luB = np.zeros((KJ_PAD, LUW), dtype=np.float32)
    luB[:NJ, :DOUT] = lup.reshape(NJ, DOUT) * SCALE            # exact x0.25
    luB[128 + 96, :DOUT] = b                                   # bias row
    cvf = np.full(KJ_PAD, -1.0, dtype=np.float32)
    cvf[:NJ] = np.arange(NJ, dtype=np.float32) // RANK
    luB[:, DOUT] = cvf                                         # compare col
    luB = np.ascontiguousarray(
        luB.reshape(2, P, LUW).transpose(1, 0, 2).astype(bf))

    in_maps = []
    for c in range(N_CORES):
        sl = slice(c * BPC, (c + 1) * BPC)
        xs = x[sl].reshape(TOK, DIN)
        eds = ed[sl].reshape(TOK, DIN)
        idxs = idx[sl].reshape(TOK).astype(np.float32)
        xTk = xs.T.reshape(KD, P, TOK).transpose(1, 0, 2).astype(bf)
        xwc = np.concatenate([xTk, WTk], axis=2)               # [128,6,1896]
        edk = np.ascontiguousarray(
            eds.T.reshape(KD, P, TOK).transpose(1, 0, 2).astype(bf))
        in_maps.append({
            "xw": np.ascontiguousarray(xwc),
            "edT": edk,
            "idxf": np.ascontiguousarray(idxs.reshape(1, TOK).astype(bf)),
            "ldT": ldT,
            "luB": luB,
        })
    return in_maps


def kernel(x, edit_direction, concept_idx, lora_down, lora_up, W, b_lin,
           _trace=False, _out_f32=False, **_ignored):
    nc = get_bass(_out_f32)
    in_maps = make_in_maps(x, edit_direction, concept_idx, lora_down,
                           lora_up, W, b_lin)
    res = run_bass_kernel_spmd(nc, in_maps, core_ids=list(range(N_CORES)),
                               trace=_trace)
    out = np.concatenate(
        [np.asarray(r["out"], dtype=np.float32) for r in res.results], axis=0)
    out = out.reshape(B, T, DOUT)
    if _trace:
        kernel.last_results = res
    return out


# revision 38
# speedup vs baseline: 1.0501x; 1.0493x over previous
"""Trainium2 Bass kernel for CPELayer_ResAG (concept-routed LoRA edit layer).

Computation (per token t with concept c = concept_idx[t]):
    down = edit_direction[t] @ lora_down[c]          # [768]@[768,4] -> [4]
    up   = down @ lora_up[c]                         # [4]@[4,1280]  -> [1280]
    out  = x[t] @ W.T + b_lin + 0.25 * up

Strategy: data-parallel over batch across 8 cores (616 tokens/core).
The routed LoRA is computed densely: A.T[(c,r), t] = lora_down_flat.T @ ed.T
for ALL concepts (only ~6% extra PE work), then masked on-device with a
one-hot built by DVE is_equal (the MoE routing), and contracted back with
lora_up_flat via the tensor engine, accumulating into the same PSUM as the
org matmul.  The bias is folded in as one extra contraction row (ones row in
the masked operand, b_lin row in the lora_up operand).  The 0.25 LoRA scale
is folded into lora_up host-side (exact: power of two).

v2 (all-bf16, overlap-tuned): every operand and the output travel as bf16
(abs rel err ~4e-3, budget 2e-2), halving HBM traffic vs fp32r. x.T and W.T
are packed host-side into one [128, 6, 616+1280] tensor so each k-tile of
the d_in contraction arrives as ONE contiguous-line DMA and the org matmuls
track per-k arrival.  A dozen dummy matmuls on a memset tile run during the
load phase so the PE clock (DVFS: 0.65/1.2/2.4 GHz ramp) is at full rate
when real work starts.  DMA issues are split across the two hardware DGE
queues (sync + scalar).  The concept-id compare column rides in the lora_up
tensor (col 1280) instead of a separate DMA.
"""

import sys
import types

import numpy as np

import concourse.mybir as mybir
import concourse.tile as tile
from concourse import bacc
from concourse.bass_utils import run_bass_kernel_spmd

# If BASS_TRACE is set in the environment, run_bass_kernel_spmd imports
# antenv.axon_hooks, which some containers lack; stub it (None hook ->
# tracing is skipped gracefully, execution unaffected).
try:
    import antenv.axon_hooks  # noqa: F401
except ImportError:
    _m = types.ModuleType("antenv.axon_hooks")
    _m.get_axon_ntff_profile_hook = lambda: None
    _m.set_axon_ntff_profile_hook = lambda h: None
    sys.modules["antenv.axon_hooks"] = _m

# Problem shapes (hardcoded per spec nn_CPELayer_ResAG_19335942766951)
N_CORES = 8
B, T, DIN, DOUT = 64, 77, 768, 1280
N_CONCEPTS, RANK = 50, 4
SCALE = 0.25  # alpha/rank = 1/4, exact power of two
BPC = B // N_CORES          # batches per core = 8
TOK = BPC * T               # tokens per core = 616
NJ = N_CONCEPTS * RANK      # 200 flattened (concept, rank) rows
KJ_PAD = 256                # padded rows: 200 lora + 1 bias + 55 zero
P = 128
KD = DIN // P               # 6 k-tiles of the d_in contraction
KH = KD // 2                # ed arrives in two halves of 3 k-tiles
NH = 308                    # half of TOK for the A.T psum tiles
XW = TOK + DOUT             # combined x.T | W.T free width = 1896
LUW = DOUT + 1              # lora_up width + concept-id compare column
T_EDGES = [0, 128, 256, 384, 512, 616]
N_CHUNKS = [(0, 512), (512, 512), (1024, 256)]
N_WARM = 24                 # dummy matmuls to ramp the PE clock; sized so
                            # the warm block ends right as ed_a/ld land
                            # (any idle gap resets the DVFS ramp, and the
                            # later warms run 2x faster once it hits full)
KA = 2                      # org k-tiles joining the up-matmuls in wave A

_cache = {}


def _build_bass(out_f32=False):
    nc = bacc.Bacc("TRN2", target_bir_lowering=False, debug=False,
                   num_devices=N_CORES)
    f32 = mybir.dt.float32
    bf16 = mybir.dt.bfloat16
    odt = f32 if out_f32 else bf16

    xw_d = nc.dram_tensor("xw", [P, KD, XW], bf16, kind="ExternalInput").ap()
    edT_d = nc.dram_tensor("edT", [P, KD, TOK], bf16,
                           kind="ExternalInput").ap()
    idx_d = nc.dram_tensor("idxf", [1, TOK], bf16, kind="ExternalInput").ap()
    ldT_d = nc.dram_tensor("ldT", [P, KD, NJ], bf16,
                           kind="ExternalInput").ap()
    lu_d = nc.dram_tensor("luB", [P, 2, LUW], bf16, kind="ExternalInput").ap()
    out_d = nc.dram_tensor("out", [TOK, DOUT], odt, kind="ExternalOutput").ap()

    with tile.TileContext(nc) as tc:
        with (
            tc.tile_pool(name="consts", bufs=1) as consts,
            tc.tile_pool(name="osb32", bufs=5) as osb32p,
            tc.tile_pool(name="osbbf", bufs=5) as osbbfp,
        ):
            # Warm-up source: zeros tile the dummy matmuls stream over while
            # the first DMAs are in flight (PE DVFS ramps after ~3us busy).
            wsrc = consts.tile([P, NH], bf16, tag="wsrc")
            nc.gpsimd.memset(wsrc[:], 0.0)

            # MT holds the masked (routed) A.T rows.  Chunk-1 rows 72..127
            # pair with luB rows 200..255: rows 64..128 zeroed, then the
            # ones row at 96 (bias: b_lin sits at luB row 224); the mask-mul
            # below overwrites rows 0..71 (lora j=128..199).
            MT = []
            for jc in (0, 1):
                mt_t = consts.tile([P, TOK], bf16, tag=f"MT{jc}")
                MT.append(mt_t)
            nc.gpsimd.memset(MT[1][64:P, :], 0.0)
            nc.gpsimd.memset(MT[1][96:97, :], 1.0)

            # DMA issues: ~0.7us each of engine-queue time, so they are
            # split across the two hardware DGE queues (sync + scalar) and
            # ordered by need: the A.T chain (ed/ld) first on sync, the
            # small routing tensors on scalar, then the org k-tiles.  idx
            # goes over as one 2.4KB line and is partition-broadcast by the
            # otherwise-idle gpsimd so it doesn't steal HBM bandwidth from
            # ed_a during the critical load window.
            # Bulk inputs ride the sync ring in need-order; the small idx
            # broadcast rides the scalar ring concurrently so the mask
            # chain (idx -> is_equal -> MT mult) completes before the A.T
            # matmuls do, never gating wave A.
            # Ring throughput scales with packet size (per-packet latency
            # ~0.2us, ~16 in flight), so ed keeps its 3696B-run layout and
            # the sync ring carries ONLY the big-packet tensor-engine
            # stream in need-order (ed_a, ed_b, xw k-tiles) while the
            # small routing tensors (ld/idx/lu) ride the scalar ring in
            # parallel.
            idx_bc = consts.tile([P, TOK], bf16, tag="idx_bc")
            nc.scalar.dma_start(idx_bc[:], idx_d.partition_broadcast(P))
            ed_a = consts.tile([P, KH, TOK], bf16, tag="ed_a")
            nc.sync.dma_start(ed_a[:], edT_d[:, 0:KH, :])
            ld_all = consts.tile([P, KD, NJ], bf16, tag="ld_all")
            nc.sync.dma_start(ld_all[:], ldT_d[:, :, :])
            ed_b = consts.tile([P, KD - KH, TOK], bf16, tag="ed_b")
            nc.sync.dma_start(ed_b[:], edT_d[:, KH:KD, :])
            lu_all = consts.tile([P, 2, LUW], bf16, tag="lu_all")
            nc.sync.dma_start(lu_all[:], lu_d[:, :, :])
            xw = []
            for k in range(KD):
                t_ = consts.tile([P, XW], bf16, tag=f"xw{k}")
                nc.sync.dma_start(t_[:], xw_d[:, k, :])
                xw.append(t_)

            edT = ([ed_a[:, k, :] for k in range(KH)]
                   + [ed_b[:, k, :] for k in range(KD - KH)])
            xT = [xw[k][:, 0:TOK] for k in range(KD)]
            WT = [xw[k][:, TOK:XW] for k in range(KD)]
            lu = [lu_all[:, j, 0:DOUT] for j in range(2)]
            # is_equal needs an f32 per-partition scalar; the compare column
            # rides in the bf16 lu tensor, so up-convert the 2 values first.
            cv32 = consts.tile([P, 2], f32, tag="cv32")
            nc.vector.tensor_copy(out=cv32[:], in_=lu_all[:, :, DOUT])
            cv = [cv32[:, j:j + 1] for j in range(2)]

            # One-hot routing masks, built by DVE while DMAs stream.
            masks = []
            for jc in range(2):
                m = consts.tile([P, TOK], f32, tag=f"mask{jc}")
                nc.vector.tensor_scalar(
                    m[:], idx_bc[:], cv[jc], None, mybir.AluOpType.is_equal)
                masks.append(m)

            with tc.tile_pool(name="at_ps", bufs=4, space="PSUM") as at_pool, \
                    tc.tile_pool(name="out_ps", bufs=4,
                                 space="PSUM") as out_pool:
                # PE clock warm-up: ~3us of zero matmuls on one psum bank
                # (shares the "at" tag/slots; it retires before the 5th
                # at-tile allocation needs the slot back).
                warm = at_pool.tile([P, NH], f32, tag="at")
                for _ in range(N_WARM):
                    nc.tensor.matmul(warm[:], wsrc[:, 0:P], wsrc[:],
                                     start=True, stop=True)

                # A.T[(c,r), t] = lora_down_flat.T @ ed.T for all concepts,
                # k-outer so k0..2 stream on ed_a while ed_b is in flight;
                # masked into MT (the routed "down" activations, transposed).
                ats = {}
                for jc in range(2):
                    jp = P if jc == 0 else NJ - P  # 128, 72
                    for nh in range(2):
                        at_t = at_pool.tile([P, NH], f32, tag="at")
                        ats[(jc, nh)] = at_t
                midwarm = out_pool.tile([P, 512], f32, tag="ops")
                for k in range(KD):
                    if k == KH:
                        for _ in range(3):
                            nc.tensor.matmul(midwarm[:, 0:NH],
                                             wsrc[:, 0:P], wsrc[:],
                                             start=True, stop=True)
                    for jc in range(2):
                        jp = P if jc == 0 else NJ - P
                        jsl = slice(jc * P, jc * P + jp)
                        for nh in range(2):
                            nsl = slice(nh * NH, (nh + 1) * NH)
                            nc.tensor.matmul(
                                ats[(jc, nh)][:jp, :], ld_all[:, k, jsl],
                                edT[k][:, nsl], start=(k == 0),
                                stop=(k == KD - 1))
                for nh in range(2):
                    nsl = slice(nh * NH, (nh + 1) * NH)
                    for jc in range(2):
                        jp = P if jc == 0 else NJ - P
                        nc.vector.tensor_tensor(
                            MT[jc][:jp, nsl], ats[(jc, nh)][:jp, :],
                            masks[jc][:jp, nsl], mybir.AluOpType.mult)

                # Main accumulation, two short-lived PSUM waves per (t, n) so
                # banks recycle while the xw k-tiles stream in.  Wave A only
                # needs MT/lu (+ xw k<KA), which arrive early; it buys ~6us
                # of PE time during which the remaining k-tiles stream in,
                # so wave B never waits on the DMA ring:
                #   wave A: up1+up2 + org k<KA -> copy to osb32
                #   wave B: org k=KA..5 -> DVE-add (bf16 out) -> DMA out
                osb32s = []
                gi = 0
                for ti in range(len(T_EDGES) - 1):
                    t0, t1 = T_EDGES[ti], T_EDGES[ti + 1]
                    tw = t1 - t0
                    tsl = slice(t0, t1)
                    osb = osb32p.tile([P, DOUT], f32, tag="osb32")
                    osb32s.append(osb)
                    for (n0, nw) in N_CHUNKS:
                        ps = out_pool.tile([P, 512], f32, tag="ops")
                        nmm = 2 + KA
                        i = 0
                        for jc in range(2):
                            nc.tensor.matmul(
                                ps[:tw, :nw], MT[jc][:, tsl],
                                lu[jc][:, n0:n0 + nw],
                                start=(i == 0), stop=(i == nmm - 1))
                            i += 1
                        for k in range(KA):
                            nc.tensor.matmul(
                                ps[:tw, :nw], xT[k][:, tsl],
                                WT[k][:, n0:n0 + nw],
                                start=(i == 0), stop=(i == nmm - 1))
                            i += 1
                        # alternate the copy engine so psum banks recycle
                        # at matmul pace
                        if gi % 2 == 0:
                            nc.scalar.copy(osb[:tw, n0:n0 + nw],
                                           ps[:tw, :nw])
                        else:
                            nc.vector.tensor_copy(out=osb[:tw, n0:n0 + nw],
                                                  in_=ps[:tw, :nw])
                        gi += 1
                # wave B runs the t-tiles in reverse so the short 104-row
                # tile drains early and the LAST tile's store is split per
                # n-chunk across both rings, minimizing the final chain
                n_t = len(T_EDGES) - 1
                for wi, ti in enumerate(reversed(range(n_t))):
                    t0, t1 = T_EDGES[ti], T_EDGES[ti + 1]
                    tw = t1 - t0
                    tsl = slice(t0, t1)
                    osb = osb32s[ti]
                    obf = osbbfp.tile([P, DOUT], odt, tag="osbbf")
                    last = (wi == n_t - 1)
                    for ni, (n0, nw) in enumerate(N_CHUNKS):
                        ps = out_pool.tile([P, 512], f32, tag="ops")
                        for i, k in enumerate(range(KA, KD)):
                            nc.tensor.matmul(
                                ps[:tw, :nw], xT[k][:, tsl],
                                WT[k][:, n0:n0 + nw],
                                start=(i == 0), stop=(i == KD - KA - 1))
                        nc.vector.tensor_tensor(
                            obf[:tw, n0:n0 + nw], ps[:tw, :nw],
                            osb[:tw, n0:n0 + nw], mybir.AluOpType.add)
                        if last:
                            oeng = nc.scalar if ni % 2 == 0 else nc.sync
                            oeng.dma_start(out_d[tsl, n0:n0 + nw],
                                           obf[:tw, n0:n0 + nw])
                    if not last:
                        # out tiles alternate rings so the ~2us-per-tile
                        # store stream runs 2-wide under the compute
                        oeng = nc.scalar if wi % 2 == 0 else nc.sync
                        oeng.dma_start(out_d[tsl, :], obf[:tw, :])

    nc.compile()
    return nc


def get_bass(out_f32=False):
    key = bool(out_f32)
    if key not in _cache:
        _cache[key] = _build_bass(out_f32)
    return _cache[key]


def make_in_maps(x, edit_direction, concept_idx, lora_down, lora_up, W,
                 b_lin):
    """Host-side sharding + layout prep (no reference FLOPs)."""
    bf = mybir.dt.np(mybir.dt.bfloat16)
    x = np.asarray(x, dtype=np.float32)
    ed = np.asarray(edit_direction, dtype=np.float32)
    idx = np.asarray(concept_idx)
    ld = np.asarray(lora_down, dtype=np.float32)
    lup = np.asarray(lora_up, dtype=np.float32)
    W = np.asarray(W, dtype=np.float32)
    b = np.asarray(b_lin, dtype=np.float32)

    # W.T as [128, 6, 1280] (k-tiles of d_in side by side per partition)
    WTk = np.ascontiguousarray(
        W.T.reshape(KD, P, DOUT).transpose(1, 0, 2).astype(bf))
    ldT = np.ascontiguousarray(
        ld.transpose(1, 0, 2).reshape(KD, P, NJ).transpose(1, 0, 2)
        .astype(bf))
    luB = np.zeros((KJ_PAD, LUW), dtype=np.float32)
    luB[:NJ, :DOUT] = lup.reshape(NJ, DOUT) * SCALE            # exact x0.25
    luB[128 + 96, :DOUT] = b                                   # bias row
    cvf = np.full(KJ_PAD, -1.0, dtype=np.float32)
    cvf[:NJ] = np.arange(NJ, dtype=np.float32) // RANK
    luB[:, DOUT] = cvf                                         # compare col
    luB = np.ascontiguousarray(
        luB.reshape(2, P, LUW).transpose(1, 0, 2).astype(bf))

    in_maps = []
    for c in range(N_CORES):
        sl = slice(c * BPC, (c + 1) * BPC)
        xs = x[sl].reshape(TOK, DIN)
        eds = ed[sl].reshape(TOK, DIN)
        idxs = idx[sl].reshape(TOK).astype(np.float32)
        xTk = xs.T.reshape(KD, P, TOK).transpose(1, 0, 2).astype(bf)
        xwc = np.concatenate([xTk, WTk], axis=2)               # [128,6,1896]
        edk = np.ascontiguousarray(
            eds.T.reshape(KD, P, TOK).transpose(1, 0, 2).astype(bf))
        in_maps.append({
            "xw": np.ascontiguousarray(xwc),
            "edT": edk,
            "idxf": np.ascontiguousarray(idxs.reshape(1, TOK).astype(bf)),
            "ldT": ldT,
            "luB": luB,
        })
    return in_maps


def kernel(x, edit_direction, concept_idx, lora_down, lora_up, W, b_lin,
           _trace=False, _out_f32=False, **_ignored):
    nc = get_bass(_out_f32)
    in_maps = make_in_maps(x, edit_direction, concept_idx, lora_down,
                           lora_up, W, b_lin)
    res = run_bass_kernel_spmd(nc, in_maps, core_ids=list(range(N_CORES)),
                               trace=_trace)
    out = np.concatenate(
        [np.asarray(r["out"], dtype=np.float32) for r in res.results], axis=0)
    out = out.reshape(B, T, DOUT)
    if _trace:
        kernel.last_results = res
    return out


# revision 39
# speedup vs baseline: 1.0601x; 1.0095x over previous
"""Trainium2 Bass kernel for CPELayer_ResAG (concept-routed LoRA edit layer).

Computation (per token t with concept c = concept_idx[t]):
    down = edit_direction[t] @ lora_down[c]          # [768]@[768,4] -> [4]
    up   = down @ lora_up[c]                         # [4]@[4,1280]  -> [1280]
    out  = x[t] @ W.T + b_lin + 0.25 * up

Strategy: data-parallel over batch across 8 cores (616 tokens/core).
The routed LoRA is computed densely: A.T[(c,r), t] = lora_down_flat.T @ ed.T
for ALL concepts (only ~6% extra PE work), then masked on-device with a
one-hot built by DVE is_equal (the MoE routing), and contracted back with
lora_up_flat via the tensor engine, accumulating into the same PSUM as the
org matmul.  The bias is folded in as one extra contraction row (ones row in
the masked operand, b_lin row in the lora_up operand).  The 0.25 LoRA scale
is folded into lora_up host-side (exact: power of two).

v2 (all-bf16, overlap-tuned): every operand and the output travel as bf16
(abs rel err ~4e-3, budget 2e-2), halving HBM traffic vs fp32r. x.T and W.T
are packed host-side into one [128, 6, 616+1280] tensor so each k-tile of
the d_in contraction arrives as ONE contiguous-line DMA and the org matmuls
track per-k arrival.  A dozen dummy matmuls on a memset tile run during the
load phase so the PE clock (DVFS: 0.65/1.2/2.4 GHz ramp) is at full rate
when real work starts.  DMA issues are split across the two hardware DGE
queues (sync + scalar).  The concept-id compare column rides in the lora_up
tensor (col 1280) instead of a separate DMA.
"""

import sys
import types

import numpy as np

import concourse.mybir as mybir
import concourse.tile as tile
from concourse import bacc
from concourse.bass_utils import run_bass_kernel_spmd

# If BASS_TRACE is set in the environment, run_bass_kernel_spmd imports
# antenv.axon_hooks, which some containers lack; stub it (None hook ->
# tracing is skipped gracefully, execution unaffected).
try:
    import antenv.axon_hooks  # noqa: F401
except ImportError:
    _m = types.ModuleType("antenv.axon_hooks")
    _m.get_axon_ntff_profile_hook = lambda: None
    _m.set_axon_ntff_profile_hook = lambda h: None
    sys.modules["antenv.axon_hooks"] = _m

# Problem shapes (hardcoded per spec nn_CPELayer_ResAG_19335942766951)
N_CORES = 8
B, T, DIN, DOUT = 64, 77, 768, 1280
N_CONCEPTS, RANK = 50, 4
SCALE = 0.25  # alpha/rank = 1/4, exact power of two
BPC = B // N_CORES          # batches per core = 8
TOK = BPC * T               # tokens per core = 616
NJ = N_CONCEPTS * RANK      # 200 flattened (concept, rank) rows
KJ_PAD = 256                # padded rows: 200 lora + 1 bias + 55 zero
P = 128
KD = DIN // P               # 6 k-tiles of the d_in contraction
KH = KD // 2                # ed arrives in two halves of 3 k-tiles
NH = 308                    # half of TOK for the A.T psum tiles
XW = TOK + DOUT             # combined x.T | W.T free width = 1896
LUW = DOUT + 1              # lora_up width + concept-id compare column
T_EDGES = [0, 128, 256, 384, 512, 616]
N_CHUNKS = [(0, 512), (512, 512), (1024, 256)]
N_WARM = 28                 # dummy matmuls to ramp the PE clock; sized so
                            # the warm block ends right as ed_a/ld land
                            # (any idle gap resets the DVFS ramp, and the
                            # later warms run 2x faster once it hits full)
KA = 2                      # org k-tiles joining the up-matmuls in wave A

_cache = {}


def _build_bass(out_f32=False):
    nc = bacc.Bacc("TRN2", target_bir_lowering=False, debug=False,
                   num_devices=N_CORES)
    f32 = mybir.dt.float32
    bf16 = mybir.dt.bfloat16
    odt = f32 if out_f32 else bf16

    xw_d = nc.dram_tensor("xw", [P, KD, XW], bf16, kind="ExternalInput").ap()
    edT_d = nc.dram_tensor("edT", [P, KD, TOK], bf16,
                           kind="ExternalInput").ap()
    idx_d = nc.dram_tensor("idxf", [1, TOK], bf16, kind="ExternalInput").ap()
    ldT_d = nc.dram_tensor("ldT", [P, KD, NJ], bf16,
                           kind="ExternalInput").ap()
    lu_d = nc.dram_tensor("luB", [P, 2, LUW], bf16, kind="ExternalInput").ap()
    out_d = nc.dram_tensor("out", [TOK, DOUT], odt, kind="ExternalOutput").ap()

    with tile.TileContext(nc) as tc:
        with (
            tc.tile_pool(name="consts", bufs=1) as consts,
            tc.tile_pool(name="osb32", bufs=5) as osb32p,
            tc.tile_pool(name="osbbf", bufs=5) as osbbfp,
        ):
            # Warm-up source: zeros tile the dummy matmuls stream over while
            # the first DMAs are in flight (PE DVFS ramps after ~3us busy).
            wsrc = consts.tile([P, NH], bf16, tag="wsrc")
            nc.gpsimd.memset(wsrc[:], 0.0)

            # MT holds the masked (routed) A.T rows.  Chunk-1 rows 72..127
            # pair with luB rows 200..255: rows 64..128 zeroed, then the
            # ones row at 96 (bias: b_lin sits at luB row 224); the mask-mul
            # below overwrites rows 0..71 (lora j=128..199).
            MT = []
            for jc in (0, 1):
                mt_t = consts.tile([P, TOK], bf16, tag=f"MT{jc}")
                MT.append(mt_t)
            nc.gpsimd.memset(MT[1][64:P, :], 0.0)
            nc.gpsimd.memset(MT[1][96:97, :], 1.0)

            # DMA issues: ~0.7us each of engine-queue time, so they are
            # split across the two hardware DGE queues (sync + scalar) and
            # ordered by need: the A.T chain (ed/ld) first on sync, the
            # small routing tensors on scalar, then the org k-tiles.  idx
            # goes over as one 2.4KB line and is partition-broadcast by the
            # otherwise-idle gpsimd so it doesn't steal HBM bandwidth from
            # ed_a during the critical load window.
            # Bulk inputs ride the sync ring in need-order; the small idx
            # broadcast rides the scalar ring concurrently so the mask
            # chain (idx -> is_equal -> MT mult) completes before the A.T
            # matmuls do, never gating wave A.
            # Ring throughput scales with packet size (per-packet latency
            # ~0.2us, ~16 in flight), so ed keeps its 3696B-run layout and
            # the sync ring carries ONLY the big-packet tensor-engine
            # stream in need-order (ed_a, ed_b, xw k-tiles) while the
            # small routing tensors (ld/idx/lu) ride the scalar ring in
            # parallel.
            idx_bc = consts.tile([P, TOK], bf16, tag="idx_bc")
            nc.scalar.dma_start(idx_bc[:], idx_d.partition_broadcast(P))
            ed_a = consts.tile([P, KH, TOK], bf16, tag="ed_a")
            nc.sync.dma_start(ed_a[:], edT_d[:, 0:KH, :])
            ld_all = consts.tile([P, KD, NJ], bf16, tag="ld_all")
            nc.sync.dma_start(ld_all[:], ldT_d[:, :, :])
            ed_b = consts.tile([P, KD - KH, TOK], bf16, tag="ed_b")
            nc.sync.dma_start(ed_b[:], edT_d[:, KH:KD, :])
            lu_all = consts.tile([P, 2, LUW], bf16, tag="lu_all")
            nc.sync.dma_start(lu_all[:], lu_d[:, :, :])
            xw = []
            for k in range(KD):
                t_ = consts.tile([P, XW], bf16, tag=f"xw{k}")
                nc.sync.dma_start(t_[:], xw_d[:, k, :])
                xw.append(t_)

            edT = ([ed_a[:, k, :] for k in range(KH)]
                   + [ed_b[:, k, :] for k in range(KD - KH)])
            xT = [xw[k][:, 0:TOK] for k in range(KD)]
            WT = [xw[k][:, TOK:XW] for k in range(KD)]
            lu = [lu_all[:, j, 0:DOUT] for j in range(2)]
            # is_equal needs an f32 per-partition scalar; the compare column
            # rides in the bf16 lu tensor, so up-convert the 2 values first.
            cv32 = consts.tile([P, 2], f32, tag="cv32")
            nc.vector.tensor_copy(out=cv32[:], in_=lu_all[:, :, DOUT])
            cv = [cv32[:, j:j + 1] for j in range(2)]

            # One-hot routing masks, built by DVE while DMAs stream.
            masks = []
            for jc in range(2):
                m = consts.tile([P, TOK], f32, tag=f"mask{jc}")
                nc.vector.tensor_scalar(
                    m[:], idx_bc[:], cv[jc], None, mybir.AluOpType.is_equal)
                masks.append(m)

            with tc.tile_pool(name="at_ps", bufs=4, space="PSUM") as at_pool, \
                    tc.tile_pool(name="out_ps", bufs=4,
                                 space="PSUM") as out_pool:
                # PE clock warm-up: ~3us of zero matmuls on one psum bank
                # (shares the "at" tag/slots; it retires before the 5th
                # at-tile allocation needs the slot back).
                warm = at_pool.tile([P, NH], f32, tag="at")
                for _ in range(N_WARM):
                    nc.tensor.matmul(warm[:], wsrc[:, 0:P], wsrc[:],
                                     start=True, stop=True)

                # A.T[(c,r), t] = lora_down_flat.T @ ed.T for all concepts,
                # masked into MT (the routed "down" activations, transposed).
                # nh-outer: the first token-half completes (and its MT mult
                # runs) while the second half is still on the tensor queue,
                # so wave A's up-matmuls start ~1us sooner.  Within a half,
                # k-outer across the 2 jc tiles so k0..2 stream on ed_a
                # while ed_b is in flight; dummy matmuls pad the ed_b wait.
                midwarm = out_pool.tile([P, 512], f32, tag="ops")
                for nh in range(2):
                    nsl = slice(nh * NH, (nh + 1) * NH)
                    ats = []
                    for jc in range(2):
                        at_t = at_pool.tile([P, NH], f32, tag="at")
                        ats.append(at_t)
                    for k in range(KD):
                        if nh == 0 and k == KH:
                            for _ in range(3):
                                nc.tensor.matmul(midwarm[:, 0:NH],
                                                 wsrc[:, 0:P], wsrc[:],
                                                 start=True, stop=True)
                        for jc in range(2):
                            jp = P if jc == 0 else NJ - P
                            jsl = slice(jc * P, jc * P + jp)
                            nc.tensor.matmul(
                                ats[jc][:jp, :], ld_all[:, k, jsl],
                                edT[k][:, nsl], start=(k == 0),
                                stop=(k == KD - 1))
                    for jc in range(2):
                        jp = P if jc == 0 else NJ - P
                        nc.vector.tensor_tensor(
                            MT[jc][:jp, nsl], ats[jc][:jp, :],
                            masks[jc][:jp, nsl], mybir.AluOpType.mult)

                # Main accumulation, two short-lived PSUM waves per (t, n) so
                # banks recycle while the xw k-tiles stream in.  Wave A only
                # needs MT/lu (+ xw k<KA), which arrive early; it buys ~6us
                # of PE time during which the remaining k-tiles stream in,
                # so wave B never waits on the DMA ring:
                #   wave A: up1+up2 + org k<KA -> copy to osb32
                #   wave B: org k=KA..5 -> DVE-add (bf16 out) -> DMA out
                osb32s = []
                gi = 0
                for ti in range(len(T_EDGES) - 1):
                    t0, t1 = T_EDGES[ti], T_EDGES[ti + 1]
                    tw = t1 - t0
                    tsl = slice(t0, t1)
                    osb = osb32p.tile([P, DOUT], f32, tag="osb32")
                    osb32s.append(osb)
                    for (n0, nw) in N_CHUNKS:
                        ps = out_pool.tile([P, 512], f32, tag="ops")
                        nmm = 2 + KA
                        i = 0
                        for jc in range(2):
                            nc.tensor.matmul(
                                ps[:tw, :nw], MT[jc][:, tsl],
                                lu[jc][:, n0:n0 + nw],
                                start=(i == 0), stop=(i == nmm - 1))
                            i += 1
                        for k in range(KA):
                            nc.tensor.matmul(
                                ps[:tw, :nw], xT[k][:, tsl],
                                WT[k][:, n0:n0 + nw],
                                start=(i == 0), stop=(i == nmm - 1))
                            i += 1
                        # alternate the copy engine so psum banks recycle
                        # at matmul pace
                        if gi % 2 == 0:
                            nc.scalar.copy(osb[:tw, n0:n0 + nw],
                                           ps[:tw, :nw])
                        else:
                            nc.vector.tensor_copy(out=osb[:tw, n0:n0 + nw],
                                                  in_=ps[:tw, :nw])
                        gi += 1
                # wave B runs the t-tiles in reverse so the short 104-row
                # tile drains early and the LAST tile's store is split per
                # n-chunk across both rings, minimizing the final chain
                n_t = len(T_EDGES) - 1
                for wi, ti in enumerate(reversed(range(n_t))):
                    t0, t1 = T_EDGES[ti], T_EDGES[ti + 1]
                    tw = t1 - t0
                    tsl = slice(t0, t1)
                    osb = osb32s[ti]
                    obf = osbbfp.tile([P, DOUT], odt, tag="osbbf")
                    last = (wi == n_t - 1)
                    for ni, (n0, nw) in enumerate(N_CHUNKS):
                        ps = out_pool.tile([P, 512], f32, tag="ops")
                        for i, k in enumerate(range(KA, KD)):
                            nc.tensor.matmul(
                                ps[:tw, :nw], xT[k][:, tsl],
                                WT[k][:, n0:n0 + nw],
                                start=(i == 0), stop=(i == KD - KA - 1))
                        nc.vector.tensor_tensor(
                            obf[:tw, n0:n0 + nw], ps[:tw, :nw],
                            osb[:tw, n0:n0 + nw], mybir.AluOpType.add)
                        if last:
                            oeng = nc.scalar if ni % 2 == 0 else nc.sync
                            oeng.dma_start(out_d[tsl, n0:n0 + nw],
                                           obf[:tw, n0:n0 + nw])
                    if not last:
                        # out tiles alternate rings so the ~2us-per-tile
                        # store stream runs 2-wide under the compute
                        oeng = nc.scalar if wi % 2 == 0 else nc.sync
                        oeng.dma_start(out_d[tsl, :], obf[:tw, :])

    nc.compile()
    return nc


def get_bass(out_f32=False):
    key = bool(out_f32)
    if key not in _cache:
        _cache[key] = _build_bass(out_f32)
    return _cache[key]


def make_in_maps(x, edit_direction, concept_idx, lora_down, lora_up, W,
                 b_lin):
    """Host-side sharding + layout prep (no reference FLOPs)."""
    bf = mybir.dt.np(mybir.dt.bfloat16)
    x = np.asarray(x, dtype=np.float32)
    ed = np.asarray(edit_direction, dtype=np.float32)
    idx = np.asarray(concept_idx)
    ld = np.asarray(lora_down, dtype=np.float32)
    lup = np.asarray(lora_up, dtype=np.float32)
    W = np.asarray(W, dtype=np.float32)
    b = np.asarray(b_lin, dtype=np.float32)

    # W.T as [128, 6, 1280] (k-tiles of d_in side by side per partition)
    WTk = np.ascontiguousarray(
        W.T.reshape(KD, P, DOUT).transpose(1, 0, 2).astype(bf))
    ldT = np.ascontiguousarray(
        ld.transpose(1, 0, 2).reshape(KD, P, NJ).transpose(1, 0, 2)
        .astype(bf))
    luB = np.zeros((KJ_PAD, LUW), dtype=np.float32)
    luB[:NJ, :DOUT] = lup.reshape(NJ, DOUT) * SCALE            # exact x0.25
    luB[128 + 96, :DOUT] = b                                   # bias row
    cvf = np.full(KJ_PAD, -1.0, dtype=np.float32)
    cvf[:NJ] = np.arange(NJ, dtype=np.float32) // RANK
    luB[:, DOUT] = cvf                                         # compare col
    luB = np.ascontiguousarray(
        luB.reshape(2, P, LUW).transpose(1, 0, 2).astype(bf))

    in_maps = []
    for c in range(N_CORES):
        sl = slice(c * BPC, (c + 1) * BPC)
        xs = x[sl].reshape(TOK, DIN)
        eds = ed[sl].reshape(TOK, DIN)
        idxs = idx[sl].reshape(TOK).astype(np.float32)
        xTk = xs.T.reshape(KD, P, TOK).transpose(1, 0, 2).astype(bf)
        xwc = np.concatenate([xTk, WTk], axis=2)               # [128,6,1896]
        edk = np.ascontiguousarray(
            eds.T.reshape(KD, P, TOK).transpose(1, 0, 2).astype(bf))
        in_maps.append({
            "xw": np.ascontiguousarray(xwc),
            "edT": edk,
            "idxf": np.ascontiguousarray(idxs.reshape(1, TOK).astype(bf)),
            "ldT": ldT,
            "luB": luB,
        })
    return in_maps


def kernel(x, edit_direction, concept_idx, lora_down, lora_up, W, b_lin,
           _trace=False, _out_f32=False, **_ignored):
    nc = get_bass(_out_f32)
    in_maps = make_in_maps(x, edit_direction, concept_idx, lora_down,
                           lora_up, W, b_lin)
    res = run_bass_kernel_spmd(nc, in_maps, core_ids=list(range(N_CORES)),
                               trace=_trace)
    out = np.concatenate(
        [np.asarray(r["out"], dtype=np.float32) for r in res.results], axis=0)
    out = out.reshape(B, T, DOUT)
    if _trace:
        kernel.last_results = res
    return out


# revision 40
# speedup vs baseline: 1.0825x; 1.0211x over previous
"""Trainium2 Bass kernel for CPELayer_ResAG (concept-routed LoRA edit layer).

Computation (per token t with concept c = concept_idx[t]):
    down = edit_direction[t] @ lora_down[c]          # [768]@[768,4] -> [4]
    up   = down @ lora_up[c]                         # [4]@[4,1280]  -> [1280]
    out  = x[t] @ W.T + b_lin + 0.25 * up

Strategy: data-parallel over batch across 8 cores (616 tokens/core).
The routed LoRA is computed densely: A.T[(c,r), t] = lora_down_flat.T @ ed.T
for ALL concepts (only ~6% extra PE work), then masked on-device with a
one-hot built by DVE is_equal (the MoE routing), and contracted back with
lora_up_flat via the tensor engine, accumulating into the same PSUM as the
org matmul.  The bias is folded in as one extra contraction row (ones row in
the masked operand, b_lin row in the lora_up operand).  The 0.25 LoRA scale
is folded into lora_up host-side (exact: power of two).

v2 (all-bf16, overlap-tuned): every operand and the output travel as bf16
(abs rel err ~4e-3, budget 2e-2), halving HBM traffic vs fp32r. x.T and W.T
are packed host-side into one [128, 6, 616+1280] tensor so each k-tile of
the d_in contraction arrives as ONE contiguous-line DMA and the org matmuls
track per-k arrival.  A dozen dummy matmuls on a memset tile run during the
load phase so the PE clock (DVFS: 0.65/1.2/2.4 GHz ramp) is at full rate
when real work starts.  DMA issues are split across the two hardware DGE
queues (sync + scalar).  The concept-id compare column rides in the lora_up
tensor (col 1280) instead of a separate DMA.
"""

import sys
import types

import numpy as np

import concourse.mybir as mybir
import concourse.tile as tile
from concourse import bacc
from concourse.bass_utils import run_bass_kernel_spmd

# If BASS_TRACE is set in the environment, run_bass_kernel_spmd imports
# antenv.axon_hooks, which some containers lack; stub it (None hook ->
# tracing is skipped gracefully, execution unaffected).
try:
    import antenv.axon_hooks  # noqa: F401
except ImportError:
    _m = types.ModuleType("antenv.axon_hooks")
    _m.get_axon_ntff_profile_hook = lambda: None
    _m.set_axon_ntff_profile_hook = lambda h: None
    sys.modules["antenv.axon_hooks"] = _m

# Problem shapes (hardcoded per spec nn_CPELayer_ResAG_19335942766951)
N_CORES = 8
B, T, DIN, DOUT = 64, 77, 768, 1280
N_CONCEPTS, RANK = 50, 4
SCALE = 0.25  # alpha/rank = 1/4, exact power of two
BPC = B // N_CORES          # batches per core = 8
TOK = BPC * T               # tokens per core = 616
NJ = N_CONCEPTS * RANK      # 200 flattened (concept, rank) rows
KJ_PAD = 256                # padded rows: 200 lora + 1 bias + 55 zero
P = 128
KD = DIN // P               # 6 k-tiles of the d_in contraction
KH = KD // 2                # ed arrives in two halves of 3 k-tiles
NH = 308                    # half of TOK for the A.T psum tiles
XW = TOK + DOUT             # combined x.T | W.T free width = 1896
LUW = DOUT + 1              # lora_up width + concept-id compare column
T_EDGES = [0, 128, 256, 384, 512, 616]
N_CHUNKS = [(0, 512), (512, 512), (1024, 256)]
N_WARM = 28                 # dummy matmuls to ramp the PE clock; sized so
                            # the warm block ends right as ed_a/ld land
                            # (any idle gap resets the DVFS ramp, and the
                            # later warms run 2x faster once it hits full)
KA = 2                      # org k-tiles joining the up-matmuls in wave A

_cache = {}


def _build_bass(out_f32=False):
    nc = bacc.Bacc("TRN2", target_bir_lowering=False, debug=False,
                   num_devices=N_CORES)
    f32 = mybir.dt.float32
    bf16 = mybir.dt.bfloat16
    odt = f32 if out_f32 else bf16

    fp8 = mybir.dt.float8e3  # e3m4: ed ~N(0,1) fits the ±15.5 range; ld is
    #                          pre-scaled x64 into it (2^-8 folded into luB)
    xw_d = nc.dram_tensor("xw", [P, KD, XW], bf16, kind="ExternalInput").ap()
    edT_d = nc.dram_tensor("edT", [P, KD, TOK], fp8,
                           kind="ExternalInput").ap()
    idx_d = nc.dram_tensor("idxf", [1, TOK], bf16, kind="ExternalInput").ap()
    ldT_d = nc.dram_tensor("ldT", [P, KD, NJ], fp8,
                           kind="ExternalInput").ap()
    lu_d = nc.dram_tensor("luB", [P, 2, LUW], bf16, kind="ExternalInput").ap()
    out_d = nc.dram_tensor("out", [TOK, DOUT], odt, kind="ExternalOutput").ap()

    with tile.TileContext(nc) as tc:
        with (
            tc.tile_pool(name="consts", bufs=1) as consts,
            tc.tile_pool(name="osb32", bufs=5) as osb32p,
            tc.tile_pool(name="osbbf", bufs=5) as osbbfp,
        ):
            # Warm-up source: zeros tile the dummy matmuls stream over while
            # the first DMAs are in flight (PE DVFS ramps after ~3us busy).
            wsrc = consts.tile([P, NH], bf16, tag="wsrc")
            nc.gpsimd.memset(wsrc[:], 0.0)

            # MT holds the masked (routed) A.T rows.  Chunk-1 rows 72..127
            # pair with luB rows 200..255: rows 64..128 zeroed, then the
            # ones row at 96 (bias: b_lin sits at luB row 224); the mask-mul
            # below overwrites rows 0..71 (lora j=128..199).
            MT = []
            for jc in (0, 1):
                mt_t = consts.tile([P, TOK], bf16, tag=f"MT{jc}")
                MT.append(mt_t)
            nc.gpsimd.memset(MT[1][64:P, :], 0.0)
            nc.gpsimd.memset(MT[1][96:97, :], 1.0)

            # DMA issues: ~0.7us each of engine-queue time, so they are
            # split across the two hardware DGE queues (sync + scalar) and
            # ordered by need: the A.T chain (ed/ld) first on sync, the
            # small routing tensors on scalar, then the org k-tiles.  idx
            # goes over as one 2.4KB line and is partition-broadcast by the
            # otherwise-idle gpsimd so it doesn't steal HBM bandwidth from
            # ed_a during the critical load window.
            # Bulk inputs ride the sync ring in need-order; the small idx
            # broadcast rides the scalar ring concurrently so the mask
            # chain (idx -> is_equal -> MT mult) completes before the A.T
            # matmuls do, never gating wave A.
            # Ring throughput scales with packet size (per-packet latency
            # ~0.2us, ~16 in flight), so ed keeps its 3696B-run layout and
            # the sync ring carries ONLY the big-packet tensor-engine
            # stream in need-order (ed_a, ed_b, xw k-tiles) while the
            # small routing tensors (ld/idx/lu) ride the scalar ring in
            # parallel.
            idx_bc = consts.tile([P, TOK], bf16, tag="idx_bc")
            nc.scalar.dma_start(idx_bc[:], idx_d.partition_broadcast(P))
            ed_a = consts.tile([P, KH, TOK], fp8, tag="ed_a")
            nc.sync.dma_start(ed_a[:], edT_d[:, 0:KH, :])
            ld_all = consts.tile([P, KD, NJ], fp8, tag="ld_all")
            nc.sync.dma_start(ld_all[:], ldT_d[:, :, :])
            ed_b = consts.tile([P, KD - KH, TOK], fp8, tag="ed_b")
            nc.sync.dma_start(ed_b[:], edT_d[:, KH:KD, :])
            lu_all = consts.tile([P, 2, LUW], bf16, tag="lu_all")
            nc.sync.dma_start(lu_all[:], lu_d[:, :, :])
            xw = []
            for k in range(KD):
                t_ = consts.tile([P, XW], bf16, tag=f"xw{k}")
                nc.sync.dma_start(t_[:], xw_d[:, k, :])
                xw.append(t_)

            edT = ([ed_a[:, k, :] for k in range(KH)]
                   + [ed_b[:, k, :] for k in range(KD - KH)])
            xT = [xw[k][:, 0:TOK] for k in range(KD)]
            WT = [xw[k][:, TOK:XW] for k in range(KD)]
            lu = [lu_all[:, j, 0:DOUT] for j in range(2)]
            # is_equal needs an f32 per-partition scalar; the compare column
            # rides in the bf16 lu tensor, so up-convert the 2 values first.
            cv32 = consts.tile([P, 2], f32, tag="cv32")
            nc.vector.tensor_copy(out=cv32[:], in_=lu_all[:, :, DOUT])
            cv = [cv32[:, j:j + 1] for j in range(2)]

            # One-hot routing masks, built by DVE while DMAs stream.
            masks = []
            for jc in range(2):
                m = consts.tile([P, TOK], f32, tag=f"mask{jc}")
                nc.vector.tensor_scalar(
                    m[:], idx_bc[:], cv[jc], None, mybir.AluOpType.is_equal)
                masks.append(m)

            with tc.tile_pool(name="at_ps", bufs=4, space="PSUM") as at_pool, \
                    tc.tile_pool(name="out_ps", bufs=4,
                                 space="PSUM") as out_pool:
                # PE clock warm-up: ~3us of zero matmuls on one psum bank
                # (shares the "at" tag/slots; it retires before the 5th
                # at-tile allocation needs the slot back).
                warm = at_pool.tile([P, NH], f32, tag="at")
                for _ in range(N_WARM):
                    nc.tensor.matmul(warm[:], wsrc[:, 0:P], wsrc[:],
                                     start=True, stop=True)

                # A.T[(c,r), t] = lora_down_flat.T @ ed.T for all concepts,
                # masked into MT (the routed "down" activations, transposed).
                # nh-outer: the first token-half completes (and its MT mult
                # runs) while the second half is still on the tensor queue,
                # so wave A's up-matmuls start ~1us sooner.  Within a half,
                # k-outer across the 2 jc tiles so k0..2 stream on ed_a
                # while ed_b is in flight; dummy matmuls pad the ed_b wait.
                midwarm = out_pool.tile([P, 512], f32, tag="ops")
                for nh in range(2):
                    nsl = slice(nh * NH, (nh + 1) * NH)
                    ats = []
                    for jc in range(2):
                        at_t = at_pool.tile([P, NH], f32, tag="at")
                        ats.append(at_t)
                    for k in range(KD):
                        if nh == 0 and k == KH:
                            for _ in range(3):
                                nc.tensor.matmul(midwarm[:, 0:NH],
                                                 wsrc[:, 0:P], wsrc[:],
                                                 start=True, stop=True)
                        for jc in range(2):
                            jp = P if jc == 0 else NJ - P
                            jsl = slice(jc * P, jc * P + jp)
                            nc.tensor.matmul(
                                ats[jc][:jp, :], ld_all[:, k, jsl],
                                edT[k][:, nsl], start=(k == 0),
                                stop=(k == KD - 1))
                    for jc in range(2):
                        jp = P if jc == 0 else NJ - P
                        nc.vector.tensor_tensor(
                            MT[jc][:jp, nsl], ats[jc][:jp, :],
                            masks[jc][:jp, nsl], mybir.AluOpType.mult)

                # Main accumulation, two short-lived PSUM waves per (t, n) so
                # banks recycle while the xw k-tiles stream in.  Wave A only
                # needs MT/lu (+ xw k<KA), which arrive early; it buys ~6us
                # of PE time during which the remaining k-tiles stream in,
                # so wave B never waits on the DMA ring:
                #   wave A: up1+up2 + org k<KA -> copy to osb32
                #   wave B: org k=KA..5 -> DVE-add (bf16 out) -> DMA out
                osb32s = []
                gi = 0
                for ti in range(len(T_EDGES) - 1):
                    t0, t1 = T_EDGES[ti], T_EDGES[ti + 1]
                    tw = t1 - t0
                    tsl = slice(t0, t1)
                    osb = osb32p.tile([P, DOUT], f32, tag="osb32")
                    osb32s.append(osb)
                    for (n0, nw) in N_CHUNKS:
                        ps = out_pool.tile([P, 512], f32, tag="ops")
                        nmm = 2 + KA
                        i = 0
                        for jc in range(2):
                            nc.tensor.matmul(
                                ps[:tw, :nw], MT[jc][:, tsl],
                                lu[jc][:, n0:n0 + nw],
                                start=(i == 0), stop=(i == nmm - 1))
                            i += 1
                        for k in range(KA):
                            nc.tensor.matmul(
                                ps[:tw, :nw], xT[k][:, tsl],
                                WT[k][:, n0:n0 + nw],
                                start=(i == 0), stop=(i == nmm - 1))
                            i += 1
                        # alternate the copy engine so psum banks recycle
                        # at matmul pace
                        if gi % 2 == 0:
                            nc.scalar.copy(osb[:tw, n0:n0 + nw],
                                           ps[:tw, :nw])
                        else:
                            nc.vector.tensor_copy(out=osb[:tw, n0:n0 + nw],
                                                  in_=ps[:tw, :nw])
                        gi += 1
                # wave B runs the t-tiles in reverse so the short 104-row
                # tile drains early and the LAST tile's store is split per
                # n-chunk across both rings, minimizing the final chain
                n_t = len(T_EDGES) - 1
                for wi, ti in enumerate(reversed(range(n_t))):
                    t0, t1 = T_EDGES[ti], T_EDGES[ti + 1]
                    tw = t1 - t0
                    tsl = slice(t0, t1)
                    osb = osb32s[ti]
                    obf = osbbfp.tile([P, DOUT], odt, tag="osbbf")
                    last = (wi == n_t - 1)
                    for ni, (n0, nw) in enumerate(N_CHUNKS):
                        ps = out_pool.tile([P, 512], f32, tag="ops")
                        for i, k in enumerate(range(KA, KD)):
                            nc.tensor.matmul(
                                ps[:tw, :nw], xT[k][:, tsl],
                                WT[k][:, n0:n0 + nw],
                                start=(i == 0), stop=(i == KD - KA - 1))
                        nc.vector.tensor_tensor(
                            obf[:tw, n0:n0 + nw], ps[:tw, :nw],
                            osb[:tw, n0:n0 + nw], mybir.AluOpType.add)
                        if last:
                            oeng = nc.scalar if ni % 2 == 0 else nc.sync
                            oeng.dma_start(out_d[tsl, n0:n0 + nw],
                                           obf[:tw, n0:n0 + nw])
                    if not last:
                        # out tiles alternate rings so the ~2us-per-tile
                        # store stream runs 2-wide under the compute
                        oeng = nc.scalar if wi % 2 == 0 else nc.sync
                        oeng.dma_start(out_d[tsl, :], obf[:tw, :])

    nc.compile()
    return nc


def get_bass(out_f32=False):
    key = bool(out_f32)
    if key not in _cache:
        _cache[key] = _build_bass(out_f32)
    return _cache[key]


def make_in_maps(x, edit_direction, concept_idx, lora_down, lora_up, W,
                 b_lin):
    """Host-side sharding + layout prep (no reference FLOPs)."""
    bf = mybir.dt.np(mybir.dt.bfloat16)
    f8 = mybir.dt.np(mybir.dt.float8e3)
    x = np.asarray(x, dtype=np.float32)
    ed = np.asarray(edit_direction, dtype=np.float32)
    idx = np.asarray(concept_idx)
    ld = np.asarray(lora_down, dtype=np.float32)
    lup = np.asarray(lora_up, dtype=np.float32)
    W = np.asarray(W, dtype=np.float32)
    b = np.asarray(b_lin, dtype=np.float32)

    # W.T as [128, 6, 1280] (k-tiles of d_in side by side per partition)
    WTk = np.ascontiguousarray(
        W.T.reshape(KD, P, DOUT).transpose(1, 0, 2).astype(bf))
    # ld rides as fp8 e3m4, pre-scaled x64 into its normal range; the
    # compensating 2^-8 (with the 0.25 LoRA scale) is folded into the
    # lora rows of luB (the bias/ones row stays unscaled).
    ldT = np.ascontiguousarray(
        (ld.transpose(1, 0, 2).reshape(KD, P, NJ) * 64.0)
        .transpose(1, 0, 2).astype(f8))
    luB = np.zeros((KJ_PAD, LUW), dtype=np.float32)
    luB[:NJ, :DOUT] = lup.reshape(NJ, DOUT) * (SCALE / 64.0)
    luB[128 + 96, :DOUT] = b                                   # bias row
    cvf = np.full(KJ_PAD, -1.0, dtype=np.float32)
    cvf[:NJ] = np.arange(NJ, dtype=np.float32) // RANK
    luB[:, DOUT] = cvf                                         # compare col
    luB = np.ascontiguousarray(
        luB.reshape(2, P, LUW).transpose(1, 0, 2).astype(bf))

    in_maps = []
    for c in range(N_CORES):
        sl = slice(c * BPC, (c + 1) * BPC)
        xs = x[sl].reshape(TOK, DIN)
        eds = ed[sl].reshape(TOK, DIN)
        idxs = idx[sl].reshape(TOK).astype(np.float32)
        xTk = xs.T.reshape(KD, P, TOK).transpose(1, 0, 2).astype(bf)
        xwc = np.concatenate([xTk, WTk], axis=2)               # [128,6,1896]
        edk = np.ascontiguousarray(
            eds.T.reshape(KD, P, TOK).transpose(1, 0, 2).astype(f8))
        in_maps.append({
            "xw": np.ascontiguousarray(xwc),
            "edT": edk,
            "idxf": np.ascontiguousarray(idxs.reshape(1, TOK).astype(bf)),
            "ldT": ldT,
            "luB": luB,
        })
    return in_maps


def kernel(x, edit_direction, concept_idx, lora_down, lora_up, W, b_lin,
           _trace=False, _out_f32=False, **_ignored):
    nc = get_bass(_out_f32)
    in_maps = make_in_maps(x, edit_direction, concept_idx, lora_down,
                           lora_up, W, b_lin)
    res = run_bass_kernel_spmd(nc, in_maps, core_ids=list(range(N_CORES)),
                               trace=_trace)
    out = np.concatenate(
        [np.asarray(r["out"], dtype=np.float32) for r in res.results], axis=0)
    out = out.reshape(B, T, DOUT)
    if _trace:
        kernel.last_results = res
    return out


# revision 43
# speedup vs baseline: 1.1614x; 1.0729x over previous
"""Trainium2 Bass kernel for CPELayer_ResAG (concept-routed LoRA edit layer).

Computation (per token t with concept c = concept_idx[t]):
    down = edit_direction[t] @ lora_down[c]          # [768]@[768,4] -> [4]
    up   = down @ lora_up[c]                         # [4]@[4,1280]  -> [1280]
    out  = x[t] @ W.T + b_lin + 0.25 * up

Strategy: data-parallel over batch across 8 cores (616 tokens/core).
The routed LoRA is computed densely: A.T[(c,r), t] = lora_down_flat.T @ ed.T,
masked on-device with a one-hot built by DVE is_equal (the MoE routing), and
contracted back with lora_up via the tensor engine, accumulating into the
same PSUM as the org matmul.  The bias rides as one extra contraction row
(ones row in the masked operand, b_lin row in the lora_up operand).

Key optimizations (all layout/precision only; every FLOP runs on device):
- all-bf16 org operands and output, fp8(e3m4) for the small LoRA-down
  branch (its contribution is ~0.6% of the output scale); rel err ~3e-3
  against the 2e-2 budget.
- x.T|W.T packed host-side into one [128, 6, 1896] tensor: each k-tile of
  the d_in contraction arrives as a single contiguous-line DMA and the org
  matmuls track per-k arrival.  Input stream rides the (fast) sync DGE
  ring in need-order; idx rides the scalar ring; outputs use both.
- dummy matmuls bridge the DMA wait so the PE DVFS ramp (0.65/1.2/2.4GHz)
  reaches full clock before real work and never drops (any queue idle gap
  resets it).
- tokens are sorted by concept per core (host-side gather; un-permuted on
  return).  With a split concept c0 whose boundary lands inside t-tile 2
  on every core, token tiles 0-1 contract only lora rows [0,4*c0) and
  tiles 3-4 only [4*c0,200): the up-matmul runs ONE 128-row pass per tile
  (two only for the boundary tile) and the all-masked-zero A.T quadrant
  (jc1, token-half 0) is skipped entirely.
"""

import sys
import types

import numpy as np

import concourse.mybir as mybir
import concourse.tile as tile
from concourse import bacc
from concourse.bass_utils import run_bass_kernel_spmd

try:
    import antenv.axon_hooks  # noqa: F401
except ImportError:
    _m = types.ModuleType("antenv.axon_hooks")
    _m.get_axon_ntff_profile_hook = lambda: None
    _m.set_axon_ntff_profile_hook = lambda h: None
    sys.modules["antenv.axon_hooks"] = _m

# Problem shapes (hardcoded per spec nn_CPELayer_ResAG_19335942766951)
N_CORES = 8
B, T, DIN, DOUT = 64, 77, 768, 1280
N_CONCEPTS, RANK = 50, 4
SCALE = 0.25  # alpha/rank = 1/4, exact power of two
LD_UP = 64.0  # fp8 pre-scale for lora_down; 1/64 folded into luB lora rows
BPC = B // N_CORES          # batches per core = 8
TOK = BPC * T               # tokens per core = 616
NJ = N_CONCEPTS * RANK      # 200 flattened (concept, rank) rows
P = 128
KD = DIN // P               # 6 k-tiles of the d_in contraction
KH = KD // 2                # ed arrives in two halves of 3 k-tiles
NH = 308                    # half of TOK for the A.T psum tiles
XW = TOK + DOUT             # combined x.T | W.T free width = 1896
LUW = DOUT + 1              # lora_up width + concept-id compare column
T_EDGES = [0, 128, 256, 384, 512, 616]
TB = 384                    # first token position owned by the jc1 up-pass
N_CHUNKS = [(0, 512), (512, 512), (1024, 256)]
N_WARM = 28                 # dummy matmuls sized to end as ed_a lands (an
                            # idle tensor-queue gap resets the DVFS ramp)
N_MIDWARM = 4               # fillers between A.T k2|k3 bridging ed_b
KA = 2                      # org k-tiles joining the up-matmuls in wave A
CW = 126                    # sorted-mode chunk width: 96 lora slots, the
                            # bias slot at aligned row 96, 29 more slots

_cache = {}


def _build_bass(c0, out_f32=False):
    """c0: split concept (tokens sorted; tiles 0-1 only use concepts <c0,
    tiles 3-4 only >=c0, tile 2 both).  c0=None builds the unsorted
    fallback (2 up-passes everywhere, full A.T)."""
    nc = bacc.Bacc("TRN2", target_bir_lowering=False, debug=False,
                   num_devices=N_CORES)
    f32 = mybir.dt.float32
    bf16 = mybir.dt.bfloat16
    fp8 = mybir.dt.float8e3
    odt = f32 if out_f32 else bf16

    if c0 is None:
        jps = [P, NJ - P]       # rows per up-pass chunk
        ups_for_tile = [[0, 1]] * 5
        skip_jc1_nh0 = False
        ldw = NJ
    else:
        # fixed 126-row chunks; bias sits at the 32-aligned row 96 (the
        # memset-alignment constraint) with a zero ld column under it, so
        # the mask-mult writes zeros there and the ones-memset (emitted
        # after the mults) sets the bias columns
        jps = [CW, CW]
        ups_for_tile = [[0], [0], [0, 1], [1], [1]]
        skip_jc1_nh0 = True
        ldw = 2 * CW

    xw_d = nc.dram_tensor("xw", [P, KD, XW], bf16, kind="ExternalInput").ap()
    edT_d = nc.dram_tensor("edT", [P, KD, TOK], fp8,
                           kind="ExternalInput").ap()
    idx_d = nc.dram_tensor("idxf", [1, TOK], bf16, kind="ExternalInput").ap()
    ldT_d = nc.dram_tensor("ldT", [P, KD, ldw], fp8,
                           kind="ExternalInput").ap()
    lu_d = nc.dram_tensor("luB", [P, 2, LUW], bf16, kind="ExternalInput").ap()
    out_d = nc.dram_tensor("out", [TOK, DOUT], odt, kind="ExternalOutput").ap()

    with tile.TileContext(nc) as tc:
        with (
            tc.tile_pool(name="consts", bufs=1) as consts,
            tc.tile_pool(name="osb32", bufs=5) as osb32p,
            tc.tile_pool(name="osbbf", bufs=5) as osbbfp,
        ):
            # Warm-up source for the clock-ramp matmuls.
            wsrc = consts.tile([P, NH], bf16, tag="wsrc")
            nc.gpsimd.memset(wsrc[:], 0.0)

            # MT holds the masked (routed) "down" activations, transposed,
            # one chunk per up-pass.  Each chunk carries the bias ones-row
            # for the token range whose tiles take bias from that pass:
            # jc0 -> tokens [0, TB), jc1 -> [TB, TOK).  Rows past the
            # ones-row are zeroed (garbage x lu-zeros could make NaNs).
            MT = []
            for jc in (0, 1):
                mt_t = consts.tile([P, TOK], bf16, tag=f"MT{jc}")
                MT.append(mt_t)
            if c0 is None:
                nc.gpsimd.memset(MT[1][64:P, :], 0.0)
                nc.gpsimd.memset(MT[1][96:97, :], 1.0)
            else:
                nc.gpsimd.memset(MT[0][96:P, :], 0.0)
                nc.gpsimd.memset(MT[1][96:P, :], 0.0)
                # (jc1, nh0) A.T quadrant is skipped, but the boundary
                # tile's jc1 pass reads cols [256, 308) of MT[1]: zero them
                nc.gpsimd.memset(MT[1][0:96, 256:NH], 0.0)

            # Input DMAs: sync ring carries the big tensor-engine stream in
            # need-order (~0.7us issue each); idx rides the scalar ring.
            idx_bc = consts.tile([P, TOK], bf16, tag="idx_bc")
            nc.scalar.dma_start(idx_bc[:], idx_d.partition_broadcast(P))
            ed_a = consts.tile([P, KH, TOK], fp8, tag="ed_a")
            nc.sync.dma_start(ed_a[:], edT_d[:, 0:KH, :])
            ld_all = consts.tile([P, KD, ldw], fp8, tag="ld_all")
            nc.sync.dma_start(ld_all[:], ldT_d[:, :, :])
            ed_b = consts.tile([P, KD - KH, TOK], fp8, tag="ed_b")
            nc.sync.dma_start(ed_b[:], edT_d[:, KH:KD, :])
            lu_all = consts.tile([P, 2, LUW], bf16, tag="lu_all")
            nc.sync.dma_start(lu_all[:], lu_d[:, :, :])
            xw = []
            for k in range(KD):
                t_ = consts.tile([P, XW], bf16, tag=f"xw{k}")
                nc.sync.dma_start(t_[:], xw_d[:, k, :])
                xw.append(t_)

            edT = ([ed_a[:, k, :] for k in range(KH)]
                   + [ed_b[:, k, :] for k in range(KD - KH)])
            xT = [xw[k][:, 0:TOK] for k in range(KD)]
            WT = [xw[k][:, TOK:XW] for k in range(KD)]
            lu = [lu_all[:, j, 0:DOUT] for j in range(2)]
            # is_equal needs an f32 per-partition scalar; the compare column
            # rides in the bf16 lu tensor, so up-convert the 2 values first.
            cv32 = consts.tile([P, 2], f32, tag="cv32")
            nc.vector.tensor_copy(out=cv32[:], in_=lu_all[:, :, DOUT])
            cv = [cv32[:, j:j + 1] for j in range(2)]

            # One-hot routing masks, built by DVE while DMAs stream.
            masks = []
            for jc in range(2):
                m = consts.tile([P, TOK], f32, tag=f"mask{jc}")
                nc.vector.tensor_scalar(
                    m[:], idx_bc[:], cv[jc], None, mybir.AluOpType.is_equal)
                masks.append(m)

            with tc.tile_pool(name="at_ps", bufs=4, space="PSUM") as at_pool, \
                    tc.tile_pool(name="out_ps", bufs=4,
                                 space="PSUM") as out_pool:
                # PE clock warm-up (shares the "at" tag/slots).
                warm = at_pool.tile([P, NH], f32, tag="at")
                for _ in range(N_WARM):
                    nc.tensor.matmul(warm[:], wsrc[:, 0:P], wsrc[:],
                                     start=True, stop=True)

                # A.T[(c,r), t]: nh-outer so the first token-half's MT is
                # ready (mask-mult done) while the second half still runs
                # on the tensor queue; k-outer within a half so k0..2
                # stream on ed_a while ed_b flies; dummy matmuls pad the
                # ed_b wait.  ld column ranges follow the c0 split.
                jsl0 = [slice(0, jps[0]), slice(jps[0], ldw)]
                midwarm = out_pool.tile([P, 512], f32, tag="ops")
                for nh in range(2):
                    nsl = slice(nh * NH, (nh + 1) * NH)
                    jcs = [0] if (skip_jc1_nh0 and nh == 0) else [0, 1]
                    ats = {}
                    for jc in jcs:
                        at_t = at_pool.tile([P, NH], f32, tag="at")
                        ats[jc] = at_t
                    for k in range(KD):
                        if nh == 0 and k == KH:
                            for _ in range(N_MIDWARM):
                                nc.tensor.matmul(midwarm[:, 0:NH],
                                                 wsrc[:, 0:P], wsrc[:],
                                                 start=True, stop=True)
                        for jc in jcs:
                            jp = jps[jc]
                            nc.tensor.matmul(
                                ats[jc][:jp, :], ld_all[:, k, jsl0[jc]],
                                edT[k][:, nsl], start=(k == 0),
                                stop=(k == KD - 1))
                    for jc in jcs:
                        jp = jps[jc]
                        nc.vector.tensor_tensor(
                            MT[jc][:jp, nsl], ats[jc][:jp, :],
                            masks[jc][:jp, nsl], mybir.AluOpType.mult)
                if c0 is not None:
                    # bias ones-rows, overwriting the mult's zeros at row
                    # 96; each chunk biases only its own tiles' tokens
                    nc.gpsimd.memset(MT[0][96:97, 0:TB], 1.0)
                    nc.gpsimd.memset(MT[1][96:97, TB:TOK], 1.0)

                # Main accumulation, two short-lived PSUM waves per (t, n):
                #   wave A: up pass(es) + org k<KA -> copy to osb32
                #   wave B: org k=KA..5 -> DVE-add (bf16 out) -> DMA out
                # Wave A only needs MT/lu (+ early xw k-tiles), buying PE
                # time while the remaining k-tiles stream in.
                osb32s = []
                gi = 0
                for ti in range(len(T_EDGES) - 1):
                    t0, t1 = T_EDGES[ti], T_EDGES[ti + 1]
                    tw = t1 - t0
                    tsl = slice(t0, t1)
                    osb = osb32p.tile([P, DOUT], f32, tag="osb32")
                    osb32s.append(osb)
                    ups = ups_for_tile[ti]
                    for (n0, nw) in N_CHUNKS:
                        ps = out_pool.tile([P, 512], f32, tag="ops")
                        nmm = len(ups) + KA
                        i = 0
                        for jc in ups:
                            nc.tensor.matmul(
                                ps[:tw, :nw], MT[jc][:, tsl],
                                lu[jc][:, n0:n0 + nw],
                                start=(i == 0), stop=(i == nmm - 1))
                            i += 1
                        for k in range(KA):
                            nc.tensor.matmul(
                                ps[:tw, :nw], xT[k][:, tsl],
                                WT[k][:, n0:n0 + nw],
                                start=(i == 0), stop=(i == nmm - 1))
                            i += 1
                        # alternate copy engines so banks recycle at mm pace
                        if gi % 2 == 0:
                            nc.scalar.copy(osb[:tw, n0:n0 + nw],
                                           ps[:tw, :nw])
                        else:
                            nc.vector.tensor_copy(out=osb[:tw, n0:n0 + nw],
                                                  in_=ps[:tw, :nw])
                        gi += 1
                # wave B runs the t-tiles in reverse so the short 104-row
                # tile drains early; the LAST tile's store is split per
                # n-chunk across both rings, minimizing the final chain.
                n_t = len(T_EDGES) - 1
                for wi, ti in enumerate(reversed(range(n_t))):
                    t0, t1 = T_EDGES[ti], T_EDGES[ti + 1]
                    tw = t1 - t0
                    tsl = slice(t0, t1)
                    osb = osb32s[ti]
                    obf = osbbfp.tile([P, DOUT], odt, tag="osbbf")
                    last = (wi == n_t - 1)
                    for ni, (n0, nw) in enumerate(N_CHUNKS):
                        ps = out_pool.tile([P, 512], f32, tag="ops")
                        for i, k in enumerate(range(KA, KD)):
                            nc.tensor.matmul(
                                ps[:tw, :nw], xT[k][:, tsl],
                                WT[k][:, n0:n0 + nw],
                                start=(i == 0), stop=(i == KD - KA - 1))
                        nc.vector.tensor_tensor(
                            obf[:tw, n0:n0 + nw], ps[:tw, :nw],
                            osb[:tw, n0:n0 + nw], mybir.AluOpType.add)
                        if last:
                            oeng = nc.scalar if ni % 2 == 0 else nc.sync
                            oeng.dma_start(out_d[tsl, n0:n0 + nw],
                                           obf[:tw, n0:n0 + nw])
                    if not last:
                        oeng = nc.scalar if wi % 2 == 0 else nc.sync
                        oeng.dma_start(out_d[tsl, :], obf[:tw, :])

    nc.compile()
    return nc


def get_bass(c0, out_f32=False):
    key = (c0, bool(out_f32))
    if key not in _cache:
        _cache[key] = _build_bass(c0, out_f32)
    return _cache[key]


def _pick_c0(idx):
    """Split concept c0 s.t. on EVERY core the first sorted position with
    concept >= c0 lies in [NH, TB] (boundary inside t-tile 2, and the
    (jc1, nh0) A.T quadrant empty).  None if no such c0 exists."""
    counts = np.stack([np.bincount(idx[c * BPC:(c + 1) * BPC].reshape(-1),
                                   minlength=N_CONCEPTS)
                       for c in range(N_CORES)])
    cum = np.cumsum(counts, axis=1)  # cum[c, v] = #tokens with concept <= v
    best, best_m = None, -1
    for c0 in range(25, 32):         # both chunks must fit 125 lora rows
        a = cum[:, c0 - 1]           # first position with concept >= c0
        m = min(int((a - NH).min()), int((TB - a).min()))
        if m >= 0 and m > best_m:
            best, best_m = int(c0), m
    return best


def make_in_maps(x, edit_direction, concept_idx, lora_down, lora_up, W,
                 b_lin, c0):
    """Host-side sharding + layout prep (gather/transpose/cast only)."""
    bf = mybir.dt.np(mybir.dt.bfloat16)
    f8 = mybir.dt.np(mybir.dt.float8e3)
    x = np.asarray(x, dtype=np.float32)
    ed = np.asarray(edit_direction, dtype=np.float32)
    idx = np.asarray(concept_idx)
    ld = np.asarray(lora_down, dtype=np.float32)
    lup = np.asarray(lora_up, dtype=np.float32)
    W = np.asarray(W, dtype=np.float32)
    b = np.asarray(b_lin, dtype=np.float32)

    WTk = np.ascontiguousarray(
        W.T.reshape(KD, P, DOUT).transpose(1, 0, 2).astype(bf))
    # lora_down as fp8 e3m4, pre-scaled x64 into its normal range; the
    # compensating 1/64 (with the 0.25 LoRA scale) is folded into the
    # lora rows of luB (the bias/ones row stays unscaled).
    ld_flat = ld.transpose(1, 0, 2).reshape(DIN, NJ) * LD_UP   # [768, 200]
    lu_flat = lup.reshape(NJ, DOUT) * (SCALE / LD_UP)
    luB = np.zeros((2, P, LUW), dtype=np.float32)
    if c0 is None:
        ldc = ld_flat
        luB[0, :P, :DOUT] = lu_flat[:P]
        luB[1, :NJ - P, :DOUT] = lu_flat[P:]
        luB[1, 96, :DOUT] = b
        luB[0, :, DOUT] = np.arange(P) // RANK
        cvb = np.full(P, -1.0)
        cvb[:NJ - P] = P // RANK + np.arange(NJ - P) // RANK
        luB[1, :, DOUT] = cvb
    else:
        # two fixed 126-wide chunks; lora slots 0..95 and 97..125, the
        # bias at slot 96 (zero column in ld so the masked A.T row is 0)
        ldc = np.zeros((DIN, 2 * CW), dtype=np.float32)
        for jc in range(2):
            lo = 0 if jc == 0 else 4 * c0
            L = 4 * c0 if jc == 0 else NJ - 4 * c0
            sl1 = min(L, 96)
            cvb = np.full(P, -1.0)
            ldc[:, jc * CW:jc * CW + sl1] = ld_flat[:, lo:lo + sl1]
            luB[jc, :sl1, :DOUT] = lu_flat[lo:lo + sl1]
            cvb[:sl1] = (lo + np.arange(sl1)) // RANK
            if L > 96:
                ldc[:, jc * CW + 97:jc * CW + 1 + L] =                     ld_flat[:, lo + 96:lo + L]
                luB[jc, 97:1 + L, :DOUT] = lu_flat[lo + 96:lo + L]
                cvb[97:1 + L] = (lo + np.arange(96, L)) // RANK
            luB[jc, 96, :DOUT] = b                       # bias row
            luB[jc, :, DOUT] = cvb
    ldT = np.ascontiguousarray(
        ldc.reshape(KD, P, ldc.shape[1]).transpose(1, 0, 2).astype(f8))
    luB = np.ascontiguousarray(luB.transpose(1, 0, 2).astype(bf))

    in_maps = []
    perms = []
    for c in range(N_CORES):
        sl = slice(c * BPC, (c + 1) * BPC)
        idxs = idx[sl].reshape(TOK)
        if c0 is None:
            perm = np.arange(TOK)
        else:
            perm = np.argsort(idxs, kind="stable")
        perms.append(perm)
        idxp = idxs[perm]
        xs = x[sl].reshape(TOK, DIN)[perm]
        eds = ed[sl].reshape(TOK, DIN)[perm]
        xTk = xs.T.reshape(KD, P, TOK).transpose(1, 0, 2).astype(bf)
        xwc = np.concatenate([xTk, WTk], axis=2)         # [128, 6, 1896]
        edk = np.ascontiguousarray(
            eds.T.reshape(KD, P, TOK).transpose(1, 0, 2).astype(f8))
        in_maps.append({
            "xw": np.ascontiguousarray(xwc),
            "edT": edk,
            "idxf": np.ascontiguousarray(
                idxp.reshape(1, TOK).astype(np.float32).astype(bf)),
            "ldT": ldT,
            "luB": luB,
        })
    return in_maps, perms


def kernel(x, edit_direction, concept_idx, lora_down, lora_up, W, b_lin,
           _trace=False, _out_f32=False, **_ignored):
    idx = np.asarray(concept_idx)
    c0 = _pick_c0(idx)
    nc = get_bass(c0, _out_f32)
    in_maps, perms = make_in_maps(x, edit_direction, concept_idx, lora_down,
                                  lora_up, W, b_lin, c0)
    res = run_bass_kernel_spmd(nc, in_maps, core_ids=list(range(N_CORES)),
                               trace=_trace)
    outs = []
    for c, r in enumerate(res.results):
        o = np.asarray(r["out"], dtype=np.float32)
        inv = np.empty_like(o)
        inv[perms[c]] = o                                # un-permute tokens
        outs.append(inv)
    out = np.concatenate(outs, axis=0).reshape(B, T, DOUT)
    if _trace:
        kernel.last_results = res
    return out
